# revision 50
# baseline (speedup 1.0000x reference)
"""Trainium2 Bass kernel for nn_DistributedDotGAT (B=32, A=100, D=10000).

Sharding: data-parallel over batch across 8 cores (4 batches/core), params
replicated. Per-core pipeline:
  A. ragged gather: DVE prefix-scan ranks + GPSIMD local_scatter compaction
     (mask/rank/idx all on DVE; coordinate decode via fmod on DVE)
  B. entry encoder (fourier features via PE + fused fmod range-reduction +
     Sin on ACT; enc bias folded into a ones feature row; enc layer-2 folded
     into comp_W1 on host) and per-agent compression with streamed bf16
     weights accumulating in PSUM
  C. 3 rounds of multi-head dot-product attention (qW^T kW folded on host so
     only one score-side projection is needed; alpha transposed via XBAR DMA)
  D. output projection: bf16 out_W prefetched during C, PSUM->SBUF batched
     copies, one batched store DMA per column chunk on the gpsimd queue
"""
import sys
import math
import numpy as np

for _p in ("/opt/trn_rl_repo", "/root/.axon_site/_ro/trn_rl_repo"):
    if _p not in sys.path:
        sys.path.insert(0, _p)

import ml_dtypes
import concourse.bass as bass
import concourse.bacc as bacc
import concourse.tile as tile
import concourse.mybir as mybir
from concourse import library_config
from concourse.bass_utils import run_bass_kernel_spmd

dt = mybir.dt
Alu = mybir.AluOpType
Act = mybir.ActivationFunctionType
Ax = mybir.AxisListType

N_CORES = 8
B, A, D = 32, 100, 10000
HID, NH, OUT, NFREQ = 256, 4, 10000, 16
E = 100          # max entries kept per (b, agent)
NGRID = 100      # row/col decode base
BL = B // N_CORES   # 4 batches per core
NPAIR = BL * A      # 400 entry columns per slot
AP_ = 112        # padded agent/partition count
STEPS = 3
TWO_PI = 2.0 * math.pi
NSLOT_CHUNK = 8
NCHUNK = AP_ // NSLOT_CHUNK          # 14 chunks of 8 slots
ENT_C = NSLOT_CHUNK * NPAIR          # 3200 entries per slot-chunk
GK = 4                               # G slots per DMA batch
OUTC = 512                           # out-proj free chunk
MAGIC = 12582912.0   # 1.5 * 2**23, fp32 round-to-int magic constant
bf16 = ml_dtypes.bfloat16


def build(debug=False):
    nc = bacc.Bacc("TRN2", target_bir_lowering=False, debug=False,
                   num_devices=N_CORES)

    def din(name, shape, dtype):
        return nc.dram_tensor(name, shape, dtype, kind="ExternalInput")

    xb = din("xb", [BL, AP_, D], dt.bfloat16)
    ptab = din("ptab", [1, D], dt.uint16)
    sliota = din("sliota", [AP_, AP_], dt.float32)
    identf = din("identf", [128, 128], dt.float32r)
    identb = din("identb", [128, 128], dt.bfloat16)
    onesrow = din("onesrow", [1, ENT_C], dt.float32r)
    dupB4 = din("dupB4", [4, 32], dt.float32r)
    addp = din("addp", [32, 1], dt.float32)
    sbp = din("sbp", [32, 1], dt.float32)
    w1pos = din("w1pos", [34, 256], dt.float32r)
    gw = din("gw", [E // GK, 128, GK, 2, 512], dt.bfloat16)
    corrv = din("corrv", [AP_, 512], dt.bfloat16)
    cb1c = din("cb1c", [128, 4], dt.float32)
    w2t = din("w2t", [128, 4, 256], dt.float32r)
    b2c = din("b2c", [128, 2], dt.float32)
    mw = din("mw", [128, NH, 2, 256], dt.float32r)
    vw = din("vw", [128, NH, 2, 256], dt.float32r)
    fpw = din("fpw", [128, 2, NH, 2, 256], dt.float16)
    fpbr = din("fpbr", [1, 2, NH, 2, 128], dt.float32r)
    lngb = din("lngb", [128, 2, 2, NH], dt.float32)
    connc = din("connc", [A, A], dt.float32)
    outwtb = din("outwtb", [128, 2, OUT], dt.float16)
    onesmat = din("onesmat", [128, 128], dt.float32r)
    lngr = din("lngr", [1, NH, 2, 128], dt.float32r)

    out = nc.dram_tensor("out", [BL, A, OUT], dt.float32, kind="ExternalOutput")

    with tile.TileContext(nc) as tc:
        nc.gpsimd.load_library(library_config.local_scatter)

        # Chain ACT ops in emission order so the scheduler cannot interleave
        # activations from different pwp table sets (each switch costs 1.3us).
        _last_act = [None]

        def act(*args, **kw):
            return nc.scalar.activation(*args, **kw)

        with tc.tile_pool(name="const", bufs=1) as cpool:
            sliota_t = cpool.tile([AP_, AP_], dt.float32, tag="sliota")
            nc.scalar.dma_start(sliota_t[:], sliota.ap())
            dupB4_t = cpool.tile([4, 32], dt.float32r, tag="dupB4")
            nc.scalar.dma_start(dupB4_t[:], dupB4.ap())
            addp_t = cpool.tile([32, 1], dt.float32, tag="addp")
            nc.scalar.dma_start(addp_t[:], addp.ap())
            sbp_t = cpool.tile([32, 1], dt.float32, tag="sbp")
            nc.scalar.dma_start(sbp_t[:], sbp.ap())
            w1pos_t = cpool.tile([34, 256], dt.float32r, tag="w1pos")
            nc.scalar.dma_start(w1pos_t[:], w1pos.ap())
            corrv_t = cpool.tile([AP_, 512], dt.bfloat16, tag="corrv")
            nc.scalar.dma_start(corrv_t[:], corrv.ap())
            cb1c_t = cpool.tile([128, 4], dt.float32, tag="cb1c")
            nc.scalar.dma_start(cb1c_t[:], cb1c.ap())
            w2t_t = cpool.tile([128, 4, 256], dt.float32r, tag="w2t")
            nc.scalar.dma_start(w2t_t[:], w2t.ap())
            b2c_t = cpool.tile([128, 2], dt.float32, tag="b2c")
            nc.scalar.dma_start(b2c_t[:], b2c.ap())
            cap_t = cpool.tile([AP_, 1], dt.float32, tag="cap")
            nc.vector.memset(cap_t[:], 255.0)
            eps_t = cpool.tile([128, 1], dt.float32, tag="eps")
            nc.vector.memset(eps_t[:], 1e-5)
            cnt_t = cpool.tile([AP_, BL], dt.float32, tag="cnt")
            # transposed entry arrays ([slot, b, pair]), persist into phase B
            # rcT packs row*256 (rc=0) and col (rc=1) for a single rcflat DMA
            rcT = cpool.tile([AP_, 2, NPAIR], dt.float32r, tag="rcT")
            valTall = cpool.tile([AP_, NPAIR], dt.float32r, tag="valT")
            invTall = cpool.tile([AP_, NPAIR], dt.bfloat16, tag="invT")
            # h ping-pong tiles (persist across phases); split per half so
            # next-step matmuls start as soon as one half is final
            htiles = [[cpool.tile([128, NPAIR], dt.float32r,
                                  name=f"hst{i}_{ih}", tag=f"hst{i}_{ih}")
                       for ih in range(2)] for i in range(2)]

            # ------------- Phase A: scan + scatter + transpose, per b -------
            with tc.tile_pool(name="pA", bufs=2) as pA, \
                 tc.tile_pool(name="pA1", bufs=1) as pA1, \
                 tc.tile_pool(name="psA", bufs=2,
                              space=bass.MemorySpace.PSUM) as psA:
                identf_t = pA1.tile([128, 128], dt.float32r, tag="identf")
                nc.scalar.dma_start(identf_t[:], identf.ap())
                identb_t = pA1.tile([128, 128], dt.bfloat16, tag="identb")
                nc.scalar.dma_start(identb_t[:], identb.ap())
                ptab_t = pA1.tile([AP_, D], dt.uint16, tag="ptab")
                nc.gpsimd.dma_start(ptab_t[:],
                                    ptab.ap().broadcast_to([AP_, D]))
                DC = 2500
                NJ = D // DC
                for b in range(BL):
                    dvps = []
                    dpps = []
                    ct_prev = None
                    for j in range(NJ):
                        dsl = slice(j * DC, (j + 1) * DC)
                        xt = pA.tile([AP_, DC], dt.bfloat16, tag="xt", bufs=3,
                                     name="xt")
                        nc.sync.dma_start(xt[:], xb.ap()[b, :, dsl])
                        mk = pA.tile([AP_, DC], dt.bfloat16, tag="mk", bufs=3,
                                     name="mk")
                        nc.vector.tensor_scalar(mk[:], xt[:], 0.0, None,
                                                Alu.not_equal)
                        ct = pA.tile([AP_, DC], dt.bfloat16, tag="ct", bufs=3,
                                     name="ct")
                        nc.vector.tensor_tensor_scan(
                            ct[:], mk[:], cap_t[:].broadcast_to([AP_, DC]),
                            0.0 if j == 0 else ct_prev[:, DC - 1:DC],
                            Alu.add, Alu.min)
                        ct_prev = ct
                        # sg = 300*mk - 301 (ACT keeps DVE free for the scan)
                        sg = pA.tile([AP_, DC], dt.bfloat16, tag="sg", bufs=3,
                                     name="sg")
                        act(sg[:], mk[:], Act.Copy, scale=300.0, bias=-301.0)
                        # idx = ct + sg  (= ct-1 for valid, out-of-range else)
                        idx16 = pA.tile([AP_, DC], dt.int16, tag="idx", bufs=3,
                                        name="idx16")
                        nc.vector.tensor_tensor(idx16[:], ct[:], sg[:],
                                                Alu.add)
                        dvp = pA.tile([AP_, 256], dt.bfloat16, tag=f"dvp{j}",
                                      name=f"dvp{j}")
                        nc.gpsimd.local_scatter(dvp[:], xt[:], idx16[:],
                                                channels=AP_, num_elems=256,
                                                num_idxs=DC)
                        dpp = pA.tile([AP_, 256], dt.uint16, tag=f"dpp{j}",
                                      name=f"dpp{j}")
                        nc.gpsimd.local_scatter(dpp[:], ptab_t[:, dsl],
                                                idx16[:], channels=AP_,
                                                num_elems=256, num_idxs=DC)
                        dvps.append(dvp)
                        dpps.append(dpp)
                    nc.vector.tensor_copy(cnt_t[:, b:b + 1],
                                          ct_prev[:, DC - 1:DC])
                    # merge chunk scatters (disjoint support)
                    va = pA.tile([AP_, 256], dt.bfloat16, tag="va", name="va")
                    vb = pA.tile([AP_, 256], dt.bfloat16, tag="vb", name="vb")
                    nc.vector.tensor_tensor(va[:], dvps[0][:], dvps[1][:],
                                            Alu.add)
                    nc.vector.tensor_tensor(vb[:], dvps[2][:], dvps[3][:],
                                            Alu.add)
                    dval = pA.tile([AP_, 256], dt.bfloat16, tag="dval",
                                   name="dval")
                    nc.vector.tensor_tensor(dval[:], va[:], vb[:], Alu.add)
                    pa = pA.tile([AP_, 256], dt.float32, tag="pa", name="pa")
                    pb_ = pA.tile([AP_, 256], dt.float32, tag="pb", name="pb")
                    nc.vector.tensor_tensor(pa[:], dpps[0][:], dpps[1][:],
                                            Alu.add)
                    nc.vector.tensor_tensor(pb_[:], dpps[2][:], dpps[3][:],
                                            Alu.add)
                    packf = pA.tile([AP_, 256], dt.float32, tag="packf",
                                    name="packf")
                    nc.vector.tensor_tensor(packf[:], pa[:], pb_[:], Alu.add)

                    # invalid-slot grid: (slot >= count)
                    invg = pA.tile([AP_, AP_], dt.bfloat16, tag="invg",
                                   name="invg")
                    nc.vector.tensor_scalar(invg[:], sliota_t[:],
                                            cnt_t[:, b:b + 1], None, Alu.is_ge)
                    # coords decode (DVE magic round): row = rnd(packf/256-.5)
                    rowt = pA.tile([AP_, AP_], dt.float32, tag="rowt",
                                   name="rowt")
                    nc.vector.tensor_scalar(rowt[:], packf[:, :AP_],
                                            2.0 ** -8, -0.498046875,
                                            Alu.mult, Alu.add)
                    rowf = pA.tile([AP_, AP_], dt.float32r, tag="rowf",
                                   name="rowf")
                    nc.vector.tensor_scalar(rowf[:], rowt[:], MAGIC, -MAGIC,
                                            Alu.add, Alu.add)
                    colf = pA.tile([AP_, AP_], dt.float32r, tag="colf",
                                   name="colf")
                    nc.vector.scalar_tensor_tensor(colf[:], rowf[:], -256.0,
                                                   packf[:, :AP_], Alu.mult,
                                                   Alu.add)

                    # transposes -> [slot, pair] layout
                    tv = psA.tile([AP_, AP_], dt.bfloat16, tag="tv", name="tv")
                    nc.tensor.transpose(tv[:], dval[:, :AP_],
                                        identb_t[:AP_, :AP_])
                    nc.vector.tensor_copy(valTall[:, b * A:(b + 1) * A],
                                          tv[:, :A])
                    tr_ = psA.tile([AP_, AP_], dt.float32r, tag="tr",
                                   name="tr_")
                    nc.tensor.transpose(tr_[:], rowf[:], identf_t[:AP_, :AP_])
                    nc.vector.tensor_copy(rcT[:, 0, b * A:(b + 1) * A],
                                          tr_[:, :A])
                    tcl = psA.tile([AP_, AP_], dt.float32r, tag="tc",
                                   name="tcl")
                    nc.tensor.transpose(tcl[:], colf[:], identf_t[:AP_, :AP_])
                    nc.vector.tensor_copy(rcT[:, 1, b * A:(b + 1) * A],
                                          tcl[:, :A])
                    ti = psA.tile([AP_, AP_], dt.bfloat16, tag="ti", name="ti")
                    nc.tensor.transpose(ti[:], invg[:], identb_t[:AP_, :AP_])
                    nc.vector.tensor_copy(invTall[:, b * A:(b + 1) * A],
                                          ti[:, :A])

            # phase C weights: loaded once phase A's queue traffic is done
            onesmat_t = cpool.tile([128, 128], dt.float32r, tag="onesmat")
            nc.scalar.dma_start(onesmat_t[:], onesmat.ap())
            mw_t = cpool.tile([128, NH, 2, 256], dt.float32r, tag="mw")
            nc.scalar.dma_start(mw_t[:], mw.ap())
            vw_t = cpool.tile([128, NH, 2, 256], dt.float32r, tag="vw")
            nc.scalar.dma_start(vw_t[:], vw.ap())
            fpw_t = cpool.tile([128, 2, NH, 2, 256], dt.float16, tag="fpw")
            nc.scalar.dma_start(fpw_t[:], fpw.ap())
            fpbr_t = cpool.tile([1, 2, NH, 2, 128], dt.float32r, tag="fpbr")
            nc.scalar.dma_start(fpbr_t[:], fpbr.ap())
            ones4_t = cpool.tile([1, 512], dt.float32r, tag="ones4")
            nc.scalar.dma_start(ones4_t[:], onesrow.ap()[0:1, 0:512])
            lngb_t = cpool.tile([128, 2, 2, NH], dt.float32, tag="lngb")
            nc.scalar.dma_start(lngb_t[:], lngb.ap())
            connc_t = cpool.tile([A, A], dt.float32, tag="connc")
            nc.scalar.dma_start(connc_t[:], connc.ap())
            lngr_t = cpool.tile([1, NH, 2, 128], dt.float32r, tag="lngr")
            nc.scalar.dma_start(lngr_t[:], lngr.ap())

            # ------------- Phase B: entry encoder + compression -------------
            with tc.tile_pool(name="pB", bufs=3) as pB, \
                 tc.tile_pool(name="psH", bufs=1,
                              space=bass.MemorySpace.PSUM) as psH, \
                 tc.tile_pool(name="psZ", bufs=2,
                              space=bass.MemorySpace.PSUM) as psZ, \
                 tc.tile_pool(name="psP", bufs=2,
                              space=bass.MemorySpace.PSUM) as psP:
                h1ps = [psH.tile([128, NPAIR], dt.float32,
                                 name=f"h1_{mq}", tag=f"h1_{mq}")
                        for mq in range(4)]
                first = True

                chunks = {}
                rcflats = {}
                ntrCs = {}

                def rcflat_dma(c):
                    if c >= NB:
                        return
                    k0 = c * NSLOT_CHUNK
                    ksl = slice(k0, k0 + NSLOT_CHUNK)
                    rcflat = pB.tile([4, NSLOT_CHUNK, NPAIR], dt.float32r,
                                     tag="rcflat", bufs=4, name="rcflat")
                    for rr in range(4):
                        nc.sync.dma_start(rcflat[rr:rr + 1],
                                          rcT[ksl, rr % 2])
                    rcflats[c] = rcflat

                def featc_dma(c):
                    if c >= NB:
                        return
                    k0 = c * NSLOT_CHUNK
                    ksl = slice(k0, k0 + NSLOT_CHUNK)
                    featC = pB.tile([34, NSLOT_CHUNK, NPAIR], dt.float32r,
                                    tag="featC", name="featC")
                    nc.sync.dma_start(featC[32:33], valTall[ksl])
                    nc.sync.dma_start(featC[33:34], onesrow.ap())
                    chunks[c] = featC

                def proj_mm(c):
                    k0 = c * NSLOT_CHUNK
                    rcflat = rcflats.pop(c)
                    ntrC = pB.tile([32, NSLOT_CHUNK, NPAIR], dt.float16,
                                   tag="ntrC", bufs=2, name="ntrC")
                    nslot = min(NSLOT_CHUNK, E - k0)
                    for ks in range(nslot):
                        proj = psP.tile([32, NPAIR], dt.float32,
                                        tag="proj", name="proj")
                        nc.tensor.matmul(proj[:], dupB4_t[:], rcflat[:, ks, :],
                                         start=True, stop=True)
                        # magic round on DVE: u2 = rnd(proj+shift) + MAGIC
                        u2 = pB.tile([32, NPAIR], dt.float32, tag="u2",
                                     bufs=1, name="u2")
                        nc.vector.tensor_scalar(u2[:], proj[:], addp_t[:],
                                                MAGIC, Alu.add, Alu.add)
                        # ntr = rnd(proj+shift) - proj
                        nc.vector.scalar_tensor_tensor(
                            ntrC[:, ks, :], u2[:], -MAGIC, proj[:],
                            Alu.add, Alu.subtract)
                    ntrCs[c] = ntrC

                def proj_sin(c, half):
                    # sin(-2pi*ntr + 2pi*shift) = sin(2pi*(x+shift)); emitted
                    # in halves interleaved between zcomp silus so the ACT
                    # queue reaches them before the chunk boundary
                    k0 = c * NSLOT_CHUNK
                    nslot = min(NSLOT_CHUNK, E - k0)
                    h = (nslot + 1) // 2
                    lo, hi = (0, h) if half == 0 else (h, nslot)
                    if lo >= hi:
                        return
                    ntrC = ntrCs[c]
                    featC = chunks[c]
                    act(featC[:32, lo:hi, :], ntrC[:, lo:hi, :], Act.Sin,
                        scale=-TWO_PI, bias=sbp_t[:])

                def zcomp(c, mid=None):
                    """zpass for chunk c+1 interleaved with comppass for c."""
                    nonlocal first
                    nz = min(NSLOT_CHUNK, E - (c + 1) * NSLOT_CHUNK) \
                        if c + 1 < NB else 0
                    ncp = min(NSLOT_CHUNK, E - c * NSLOT_CHUNK) if c >= 0 else 0
                    sts_new = []
                    for ks in range(NSLOT_CHUNK):
                        if mid is not None and ks in (3, 6):
                            mid(0 if ks == 3 else 1)
                        if ks < nz:
                            featC = chunks[c + 1]
                            s_t = pB.tile([128, 2, NPAIR], dt.bfloat16,
                                          tag="s", bufs=2 * NSLOT_CHUNK,
                                          name="s")
                            for mh in range(2):
                                zb = psZ.tile([128, 512], dt.float32,
                                              tag="zb", name="zb")
                                nc.tensor.matmul(
                                    zb[:, :NPAIR],
                                    w1pos_t[:, mh * 128:(mh + 1) * 128],
                                    featC[:, ks, :], start=True, stop=True)
                                act(s_t[:, mh, :], zb[:, :NPAIR], Act.Silu)
                            sts_new.append(s_t)
                        # comppass for chunk c, slot ks
                        if ks < ncp:
                            k = c * NSLOT_CHUNK + ks
                            if k % GK == 0:
                                zcomp.gk4 = pB.tile([128, GK, 2, 512],
                                                    dt.bfloat16, tag="gk",
                                                    bufs=2, name="gk")
                                nc.scalar.dma_start(zcomp.gk4[:],
                                                    gw.ap()[k // GK])
                            gk4 = zcomp.gk4
                            s_t = zcomp.sts[ks]
                            for kh in range(2):
                                for mq in range(4):
                                    nc.tensor.matmul(
                                        h1ps[mq][:],
                                        gk4[:, k % GK, kh,
                                            mq * 128:(mq + 1) * 128],
                                        s_t[:, kh, :],
                                        start=first, stop=False,
                                        skip_group_check=True)
                                first = False
                    zcomp.sts = sts_new

                NB = (E + NSLOT_CHUNK - 1) // NSLOT_CHUNK
                for cc in range(4):
                    rcflat_dma(cc)
                featc_dma(0)
                featc_dma(1)
                proj_mm(0)
                proj_sin(0, 0)
                proj_sin(0, 1)
                featc_dma(2)
                zcomp(-1, mid=lambda half: (proj_mm(1) if half == 0 else None,
                                            proj_sin(1, half))
                      )     # prime: zpass chunk 0; chunk-1 skeleton mid-loop
                for c in range(NB):
                    rcflat_dma(c + 4)
                    featc_dma(c + 3)
                    if c + 2 < NB:
                        proj_mm(c + 2)
                    # comp chunk c + zpass chunk c+1; sin halves for c+2
                    # interleave mid-loop so ACT reaches them early
                    zcomp(c, mid=(lambda half, cc=c:
                                  proj_sin(cc + 2, half))
                          if c + 2 < NB else None)

                # pad-token corrections + comp layer 2
                for mq in range(4):
                    nc.tensor.matmul(h1ps[mq][:],
                                     corrv_t[:, mq * 128:(mq + 1) * 128],
                                     invTall[:], start=False, stop=True,
                                     skip_group_check=True)
                hsw = pB.tile([128, 4, NPAIR], dt.float32r, tag="hsw",
                              bufs=1)
                for mq in range(4):
                    act(hsw[:, mq, :], h1ps[mq][:], Act.Silu,
                        bias=cb1c_t[:, mq:mq + 1])
                for ih in range(2):
                    zb2 = psZ.tile([128, 512], dt.float32, tag="zb",
                                   name="zb2")
                    for kq in range(4):
                        nc.tensor.matmul(zb2[:, :NPAIR],
                                         w2t_t[:, kq, ih * 128:(ih + 1) * 128],
                                         hsw[:, kq, :],
                                         start=(kq == 0), stop=(kq == 3))
                    act(htiles[0][ih][:], zb2[:, :NPAIR], Act.Identity,
                        bias=b2c_t[:, ih:ih + 1])

            # ------------- Phase C: GAT message passing ---------------------
            with tc.tile_pool(name="pD0", bufs=1) as pD0:
                # bf16 out_W, prefetched in chunks between attention steps
                woutb = pD0.tile([128, 2, OUT], dt.float16, tag="woutb")
                WCH = OUT // 4

                def wout_prefetch(i):
                    sl = slice(i * WCH, (i + 1) * WCH)
                    nc.sync.dma_start(woutb[:, :, sl], outwtb.ap()[:, :, sl])

                phaseC = tc.tile_pool(name="pC1", bufs=1)
                pC1 = phaseC.__enter__()
                pC2 = tc.tile_pool(name="pC2", bufs=3).__enter__()
                psQ = tc.tile_pool(name="psQ", bufs=3,
                                   space=bass.MemorySpace.PSUM).__enter__()
                psS = tc.tile_pool(name="psS", bufs=2,
                                   space=bass.MemorySpace.PSUM).__enter__()

                hcur = htiles[0]
                for step in range(STEPS):
                    wout_prefetch(step)
                    # bf16 copy of h shared by all heads (scores rhs)
                    hb = pC2.tile([128, 2, NPAIR], dt.bfloat16, tag="hb",
                                  name="hb")
                    for kh in range(2):
                        nc.vector.tensor_copy(hb[:, kh, :], hcur[:, kh, :])
                    Rbs, VTs, aTs = [], [], []
                    # --- R = (qW^T kW / 16)^T-projected h;  V ---
                    for n in range(NH):
                        rp = psQ.tile([128, 2, 512], dt.float32, tag="q2",
                                      name="rp")
                        for jh in range(2):
                            for kh in range(2):
                                nc.tensor.matmul(
                                    rp[:, jh, :NPAIR],
                                    mw_t[:, n, kh, jh * 128:(jh + 1) * 128],
                                    hcur[:, kh, :],
                                    start=(kh == 0), stop=(kh == 1))
                        rb = pC1.tile([128, 2, NPAIR], dt.bfloat16,
                                      tag=f"rb{n}", name=f"rb{n}")
                        nc.vector.tensor_copy(rb[:], rp[:, :, :NPAIR])
                        Rbs.append(rb)
                        vp = psQ.tile([128, 2, 512], dt.float32, tag="q2",
                                      name="vp")
                        for b in range(BL):
                            for kh in range(2):
                                nc.tensor.matmul(
                                    vp[:A, b // 2, (b % 2) * 256:
                                       (b % 2) * 256 + 256],
                                    hcur[:, kh, b * A:(b + 1) * A],
                                    vw_t[:, n, kh, :],
                                    start=(kh == 0), stop=(kh == 1))
                        vt_t = pC1.tile([AP_, BL, 256], dt.bfloat16,
                                        tag=f"vts{n}", name=f"vts{n}")
                        nc.vector.tensor_copy(
                            vt_t[:A].reshape([A, 2, 2, 256]),
                            vp[:A].reshape([A, 2, 2, 256]))
                        VTs.append(vt_t)
                    # --- scores + softmax, batched over b per head ---
                    for n in range(NH):
                        scp = psS.tile([A, 512], dt.float32, tag="sc",
                                       name="scp")
                        for b in range(BL):
                            for kh in range(2):
                                nc.tensor.matmul(
                                    scp[:, b * 128:b * 128 + A],
                                    Rbs[n][:, kh, b * A:(b + 1) * A],
                                    hb[:, kh, b * A:(b + 1) * A],
                                    start=(kh == 0), stop=(kh == 1))
                        sca = pC2.tile([A, BL, A], dt.float32, tag="sca",
                                       name="sca")
                        nc.vector.tensor_tensor(
                            sca[:],
                            scp[:].reshape([A, BL, 128])[:, :, :A],
                            connc_t[:].reshape([A, 1, A])
                            .broadcast_to([A, BL, A]),
                            Alu.add)
                        esc = pC2.tile([A, BL, A], dt.float32, tag="esc",
                                       name="esc")
                        act(esc[:], sca[:], Act.Exp)
                        sm = pC2.tile([A, BL], dt.float32, tag="sm", name="sm")
                        nc.vector.tensor_reduce(sm[:], esc[:], Ax.X, Alu.add)
                        rs = pC2.tile([A, BL], dt.float32, tag="rs", name="rs")
                        nc.vector.reciprocal(rs[:], sm[:])
                        alp = pC2.tile([128, BL, 128], dt.bfloat16, tag="alp",
                                       name="alp")
                        nc.vector.tensor_tensor(
                            alp[:A, :, :A], esc[:],
                            rs[:].reshape([A, BL, 1]).broadcast_to([A, BL, A]),
                            Alu.mult)
                        at_t = pC1.tile([128, BL, 128], dt.bfloat16,
                                        tag=f"ats{n}", name=f"ats{n}")
                        for b in range(BL):
                            nc.sync.dma_start_transpose(at_t[:, b, :],
                                                        alp[:, b, :])
                        aTs.append(at_t)
                    # --- message + MLP + LN stats ---
                    ms, vvs, tss = [], [], []
                    for n in range(NH):
                        hmp = psQ.tile([128, 2, 512], dt.float32, tag="q2",
                                       name="hmp")
                        for jh in range(2):
                            for b in range(BL):
                                nc.tensor.matmul(
                                    hmp[:, jh, b * A:(b + 1) * A],
                                    VTs[n][:A, b, jh * 128:(jh + 1) * 128],
                                    aTs[n][:A, b, :A],
                                    start=True, stop=True,
                                    skip_group_check=True)
                        hs_t = pC2.tile([128, 2, NPAIR], dt.bfloat16,
                                        tag="hs")
                        act(hs_t[:], hmp[:, :, :NPAIR], Act.Silu)
                        t1p = psQ.tile([128, 2, 512], dt.float32, tag="q2",
                                       name="t1p")
                        for ih in range(2):
                            for jh in range(2):
                                nc.tensor.matmul(
                                    t1p[:, ih, :NPAIR],
                                    fpw_t[:, 0, n, jh, ih * 128:(ih + 1) * 128],
                                    hs_t[:, jh, :],
                                    start=(jh == 0), stop=(jh == 1))
                        t1s = pC2.tile([128, 2, NPAIR], dt.bfloat16, tag="t1s")
                        for ih in range(2):
                            act(t1s[:, ih, :], t1p[:, ih, :NPAIR], Act.Silu,
                                bias=fpb_t[:, 0, ih, n:n + 1])
                        t2p = psQ.tile([128, 2, 512], dt.float32, tag="q2",
                                       name="t2p")
                        for ih in range(2):
                            for jh in range(2):
                                nc.tensor.matmul(
                                    t2p[:, ih, :NPAIR],
                                    fpw_t[:, 1, n, jh, ih * 128:(ih + 1) * 128],
                                    t1s[:, jh, :],
                                    start=(jh == 0), stop=(jh == 1))
                        ts_t = pC1.tile([128, 2, NPAIR], dt.float32r,
                                        tag=f"ts{n}")
                        for ih in range(2):
                            act(ts_t[:, ih, :], t2p[:, ih, :NPAIR],
                                Act.Identity, bias=fpb_t[:, 1, ih, n:n + 1])
                        tsq = pC2.tile([128, 2, NPAIR], dt.float32r, tag="tsq")
                        nc.vector.tensor_tensor(tsq[:], ts_t[:], ts_t[:],
                                                Alu.mult)
                        mtp = psS.tile([A, 512], dt.float32, tag="sc",
                                       name="mtp")
                        for ih in range(2):
                            nc.tensor.matmul(mtp[:1, :NPAIR],
                                             onesmat_t[:, 0:1],
                                             ts_t[:, ih, :],
                                             start=(ih == 0), stop=(ih == 1))
                        vtp = psS.tile([A, 512], dt.float32, tag="sc",
                                       name="vtp")
                        for ih in range(2):
                            nc.tensor.matmul(vtp[:1, :NPAIR],
                                             onesmat_t[:, 0:1],
                                             tsq[:, ih, :],
                                             start=(ih == 0), stop=(ih == 1))
                        m_t = pC1.tile([1, NPAIR], dt.float32r, tag=f"m{n}")
                        act(m_t[:], mtp[:1, :NPAIR], Act.Identity,
                            scale=1.0 / 256.0)
                        msq = pC2.tile([1, NPAIR], dt.float32r, tag="msq")
                        nc.vector.tensor_tensor(msq[:], m_t[:], m_t[:],
                                                Alu.mult)
                        vv = pC1.tile([1, NPAIR], dt.float32, tag=f"vv{n}")
                        nc.vector.scalar_tensor_tensor(
                            vv[:], vtp[:1, :NPAIR], 1.0 / 256.0, msq[:],
                            Alu.mult, Alu.subtract)
                        ms.append(m_t); vvs.append(vv); tss.append(ts_t)
                    # --- rstd = exp(-0.5*ln(v+eps)) (one ln+exp table set) ---
                    rstds = []
                    lnvs = []
                    for n in range(NH):
                        lnv = pC1.tile([1, NPAIR], dt.float32, tag=f"lnv{n}",
                                       name=f"lnv{n}")
                        act(lnv[:], vvs[n][:], Act.Ln, bias=eps_t[:1])
                        lnvs.append(lnv)
                    for n in range(NH):
                        rstd = pC1.tile([1, NPAIR], dt.float32r,
                                        tag=f"rsd{n}", name=f"rsd{n}")
                        act(rstd[:], lnvs[n][:], Act.Exp, scale=-0.5)
                        rstds.append(rstd)
                    # --- hnew = sum_n ts_n*(g*rstd)_n + bsum - sum_n(g*m*rstd)
                    hnew = htiles[(step + 1) % 2]
                    mrs = []
                    for n in range(NH):
                        mr = pC1.tile([1, NPAIR], dt.float32r, tag=f"mr{n}",
                                      name=f"mr{n}")
                        nc.vector.tensor_tensor(mr[:], ms[n][:], rstds[n][:],
                                                Alu.mult)
                        mrs.append(mr)
                    for ih in range(2):
                        mgp = psQ.tile([128, 2, 512], dt.float32, tag="q2",
                                       name="mgp")
                        for n in range(NH):
                            nc.tensor.matmul(mgp[:, 0, :NPAIR],
                                             lngr_t[0:1, n, ih, :],
                                             mrs[n][:], start=(n == 0),
                                             stop=(n == 3))
                        for n in range(NH):
                            nc.tensor.matmul(mgp[:, 1, :NPAIR],
                                             lngr_t[0:1, n, ih, :],
                                             rstds[n][:], start=(n == 0),
                                             stop=(n == 3),
                                             skip_group_check=True)
                            if n == 0:
                                nc.vector.tensor_tensor(hnew[:, ih, :],
                                                        tss[n][:, ih, :],
                                                        mgp[:, 1, :NPAIR],
                                                        Alu.mult)
                            else:
                                u1 = pC2.tile([128, NPAIR], dt.float32,
                                              tag="u1", name="u1")
                                nc.vector.tensor_tensor(u1[:],
                                                        tss[n][:, ih, :],
                                                        mgp[:, 1, :NPAIR],
                                                        Alu.mult)
                                nc.vector.tensor_tensor(hnew[:, ih, :],
                                                        hnew[:, ih, :], u1[:],
                                                        Alu.add)
                        nc.vector.scalar_tensor_tensor(
                            hnew[:, ih, :], hnew[:, ih, :],
                            lngb_t[:, 1, ih, 0:1], mgp[:, 0, :NPAIR],
                            Alu.add, Alu.subtract)
                    hcur = hnew
                wout_prefetch(3)

                # ------------- Phase D: output projection -------------------
                with tc.tile_pool(name="pD", bufs=3) as pD, \
                     tc.tile_pool(name="psD", bufs=2,
                                  space=bass.MemorySpace.PSUM) as psD:
                    hfb = pD.tile([128, 2, NPAIR], dt.float16, tag="hfb",
                                  bufs=1)
                    for ih in range(2):
                        nc.vector.tensor_copy(hfb[:, ih, :], hcur[ih][:])
                    for ci, c0 in enumerate(range(0, OUT, OUTC)):
                        w = min(OUTC, OUT - c0)
                        pop = psD.tile([A, BL, OUTC], dt.float32, tag="pop")
                        for b in range(BL):
                            for ih in range(2):
                                nc.tensor.matmul(pop[:, b, :w],
                                                 hfb[:, ih, b * A:(b + 1) * A],
                                                 woutb[:, ih, c0:c0 + w],
                                                 start=(ih == 0),
                                                 stop=(ih == 1),
                                                 skip_group_check=True)
                        ost = pD.tile([A, BL, OUTC], dt.float32, tag="ost")
                        if ci % 2 == 0:
                            act(ost[:], pop[:], Act.Copy)
                        else:
                            nc.vector.tensor_copy(ost[:], pop[:])
                        nc.gpsimd.dma_start(
                            out.ap()[:, :, c0:c0 + w]
                            .rearrange("b a c -> a b c"),
                            ost[:, :, :w])

    nc.compile()
    return nc


def host_prep(inputs):
    f32 = np.float32
    x = np.asarray(inputs["x"], f32)
    enc_W1 = np.asarray(inputs["enc_W1"], f32)
    enc_b1 = np.asarray(inputs["enc_b1"], f32)
    enc_W2 = np.asarray(inputs["enc_W2"], f32)
    enc_b2 = np.asarray(inputs["enc_b2"], f32)
    comp_W1 = np.asarray(inputs["comp_W1"], f32)
    comp_b1 = np.asarray(inputs["comp_b1"], f32)
    comp_W2 = np.asarray(inputs["comp_W2"], f32)
    comp_b2 = np.asarray(inputs["comp_b2"], f32)
    pad = np.asarray(inputs["pad_token"], f32)
    fB = np.asarray(inputs["fourier_B"], f32)
    qW = np.asarray(inputs["qW"], f32)
    kW = np.asarray(inputs["kW"], f32)
    vW = np.asarray(inputs["vW"], f32)
    fp_W1 = np.asarray(inputs["fp_W1"], f32)
    fp_b1 = np.asarray(inputs["fp_b1"], f32)
    fp_W2 = np.asarray(inputs["fp_W2"], f32)
    fp_b2 = np.asarray(inputs["fp_b2"], f32)
    ln_g = np.asarray(inputs["ln_g"], f32)
    ln_b = np.asarray(inputs["ln_b"], f32)
    conn = np.asarray(inputs["connectivity"], f32)
    out_W = np.asarray(inputs["out_W"], f32)

    M = comp_W1.reshape(512, E, HID)
    G = np.einsum('rkj,jl->rkl', M, enc_W2, optimize=True)      # [512, E, 256]
    feat0 = np.concatenate([[0.0], np.zeros(16, f32),
                            np.ones(16, f32)]).astype(f32)
    z00 = feat0 @ enc_W1.T + enc_b1
    e00 = (z00 / (1 + np.exp(-z00))) @ enc_W2.T + enc_b2
    corrV = np.einsum('rkj,j->rk', M, (pad - e00))               # [512, E]
    cb1p = comp_b1 + np.einsum('rkj,j->r', M, enc_b2)

    # gw[g, p, ki, kh, r] = G[r, g*GK+ki, kh*128+p]
    Gr = G.reshape(512, E // GK, GK, 2, 128)          # [r, g, ki, kh, p]
    gw = np.ascontiguousarray(Gr.transpose(1, 4, 2, 3, 0)).astype(bf16)

    corrv = np.zeros((AP_, 512), f32)
    corrv[:E] = corrV.T
    corrv = corrv.astype(bf16)

    # split fourier coefs: 10-bit-quantized high part (integer-exact products
    # through the f32r matmul) + small residual; row features carry row*256
    # so the row coefficients are pre-divided by 256 (exact in fp32)
    bhi = np.round(fB * 1024.0) / 1024.0
    blo = (fB - bhi).astype(f32)
    bhi = bhi.astype(f32)
    dupB4 = np.zeros((4, 32), f32)
    dupB4[0, :16] = bhi[:, 0]; dupB4[0, 16:] = bhi[:, 0]
    dupB4[1, :16] = bhi[:, 1]; dupB4[1, 16:] = bhi[:, 1]
    dupB4[2, :16] = blo[:, 0]; dupB4[2, 16:] = blo[:, 0]
    dupB4[3, :16] = blo[:, 1]; dupB4[3, 16:] = blo[:, 1]
    # per-partition phase shift (+0.25 on the cos half), applied before the
    # magic-round add so it survives fp32; matching sin-arg bias
    addp = np.zeros((32, 1), f32)
    addp[16:] = 0.25
    sbp = np.zeros((32, 1), f32)
    sbp[16:] = TWO_PI * 0.25

    w1pos = np.zeros((34, 256), f32)
    w1pos[:32] = enc_W1[:, 1:33].T
    w1pos[32] = enc_W1[:, 0]
    w1pos[33] = enc_b1

    cb1c = np.ascontiguousarray(cb1p.reshape(4, 128).T)
    w2t = np.ascontiguousarray(
        comp_W2.T.reshape(4, 128, 256).transpose(1, 0, 2))
    b2c = np.ascontiguousarray(comp_b2.reshape(2, 128).T)

    # fold qW^T kW (and the 1/sqrt(HID) score scale) into one matrix
    Mn = np.einsum('nji,njk->nik', qW * 0.25, kW * 0.25)   # [NH, 256, 256]
    mw = np.ascontiguousarray(
        Mn.reshape(NH, 2, 128, 256).transpose(2, 0, 1, 3))  # [128,NH,2,256]
    vw = np.ascontiguousarray(
        vW.transpose(0, 2, 1).reshape(NH, 2, 128, 256)
        .transpose(2, 0, 1, 3))                             # [128,NH,2,256]
    fpw = np.stack([fp_W1, fp_W2])                    # [2, n, i, j]
    fpw = fpw.transpose(0, 1, 3, 2).reshape(2, NH, 2, 128, 256)
    fpw = np.ascontiguousarray(fpw.transpose(3, 0, 1, 2, 4)).astype(np.float16)
    fpbr = np.ascontiguousarray(
        np.stack([fp_b1, fp_b2]).reshape(2, NH, 2, 128))[None]
    lngb = np.zeros((128, 2, 2, NH), f32)
    lg = (ln_g / 4.0).reshape(NH, 2, 128)             # [n, ih, p]
    lngb[:, 0, :, :] = lg.transpose(2, 1, 0)
    bsum = (ln_b / 4.0).sum(0).reshape(2, 128)        # [ih, p]
    lngb[:, 1, :, 0] = bsum.T

    outwtb = np.ascontiguousarray(
        out_W.T.reshape(2, 128, OUT).transpose(1, 0, 2)).astype(np.float16)

    ptab = (np.arange(D, dtype=np.uint32) // NGRID * 256
            + np.arange(D, dtype=np.uint32) % NGRID).astype(np.uint16)
    sliota = np.ascontiguousarray(
        np.broadcast_to(np.arange(AP_, dtype=f32)[None, :], (AP_, AP_)))
    identf = np.eye(128, dtype=f32)
    identb = np.eye(128, dtype=f32).astype(bf16)

    shared = {
        "ptab": ptab[None, :], "sliota": sliota, "identf": identf,
        "identb": identb, "onesrow": np.ones((1, ENT_C), f32),
        "dupB4": dupB4, "addp": addp, "sbp": sbp, "w1pos": w1pos, "gw": gw,
        "corrv": corrv, "cb1c": cb1c, "w2t": w2t, "b2c": b2c, "mw": mw,
        "vw": vw, "fpw": fpw, "fpbr": fpbr, "lngb": lngb,
        "connc": np.ascontiguousarray(conn), "outwtb": outwtb,
        "onesmat": np.ones((128, 128), f32),
        "lngr": np.ascontiguousarray((ln_g / 4.0).reshape(NH, 2, 128))[None],
    }

    xp = np.zeros((B, AP_, D), f32)
    xp[:, :A, :] = x
    xpb = xp.astype(bf16)

    in_maps = []
    for core in range(N_CORES):
        m = dict(shared)
        m["xb"] = np.ascontiguousarray(xpb[core * BL:(core + 1) * BL])
        in_maps.append(m)
    return in_maps


_NC_CACHE = {}


def kernel(**inputs):
    if "nc" not in _NC_CACHE:
        _NC_CACHE["nc"] = build()
    nc = _NC_CACHE["nc"]
    in_maps = host_prep(inputs)
    res = run_bass_kernel_spmd(nc, in_maps, core_ids=list(range(N_CORES)))
    out = np.concatenate([r["out"] for r in res.results], axis=0)
    out = out + np.asarray(inputs["out_b"], np.float32)[None, None, :]
    return out.astype(np.float32)


# revision 51
# speedup vs baseline: 1.0003x; 1.0003x over previous
"""Trainium2 Bass kernel for nn_DistributedDotGAT (B=32, A=100, D=10000).

Sharding: data-parallel over batch across 8 cores (4 batches/core), params
replicated. Per-core pipeline:
  A. ragged gather: DVE prefix-scan ranks + GPSIMD local_scatter compaction
     (mask/rank/idx all on DVE; coordinate decode via fmod on DVE)
  B. entry encoder (fourier features via PE + fused fmod range-reduction +
     Sin on ACT; enc bias folded into a ones feature row; enc layer-2 folded
     into comp_W1 on host) and per-agent compression with streamed bf16
     weights accumulating in PSUM
  C. 3 rounds of multi-head dot-product attention (qW^T kW folded on host so
     only one score-side projection is needed; alpha transposed via XBAR DMA)
  D. output projection: bf16 out_W prefetched during C, PSUM->SBUF batched
     copies, one batched store DMA per column chunk on the gpsimd queue
"""
import sys
import math
import numpy as np

for _p in ("/opt/trn_rl_repo", "/root/.axon_site/_ro/trn_rl_repo"):
    if _p not in sys.path:
        sys.path.insert(0, _p)

import ml_dtypes
import concourse.bass as bass
import concourse.bacc as bacc
import concourse.tile as tile
import concourse.mybir as mybir
from concourse import library_config
from concourse.bass_utils import run_bass_kernel_spmd

dt = mybir.dt
Alu = mybir.AluOpType
Act = mybir.ActivationFunctionType
Ax = mybir.AxisListType

N_CORES = 8
B, A, D = 32, 100, 10000
HID, NH, OUT, NFREQ = 256, 4, 10000, 16
E = 100          # max entries kept per (b, agent)
NGRID = 100      # row/col decode base
BL = B // N_CORES   # 4 batches per core
NPAIR = BL * A      # 400 entry columns per slot
AP_ = 112        # padded agent/partition count
STEPS = 3
TWO_PI = 2.0 * math.pi
NSLOT_CHUNK = 8
NCHUNK = AP_ // NSLOT_CHUNK          # 14 chunks of 8 slots
ENT_C = NSLOT_CHUNK * NPAIR          # 3200 entries per slot-chunk
GK = 4                               # G slots per DMA batch
OUTC = 512                           # out-proj free chunk
MAGIC = 12582912.0   # 1.5 * 2**23, fp32 round-to-int magic constant
bf16 = ml_dtypes.bfloat16


def build(debug=False):
    nc = bacc.Bacc("TRN2", target_bir_lowering=False, debug=False,
                   num_devices=N_CORES)

    def din(name, shape, dtype):
        return nc.dram_tensor(name, shape, dtype, kind="ExternalInput")

    xb = din("xb", [BL, AP_, D], dt.bfloat16)
    ptab = din("ptab", [1, D], dt.uint16)
    sliota = din("sliota", [AP_, AP_], dt.float32)
    identf = din("identf", [128, 128], dt.float32r)
    identb = din("identb", [128, 128], dt.bfloat16)
    onesrow = din("onesrow", [1, ENT_C], dt.float32r)
    dupB4 = din("dupB4", [4, 32], dt.float32r)
    addp = din("addp", [32, 1], dt.float32)
    sbp = din("sbp", [32, 1], dt.float32)
    w1pos = din("w1pos", [34, 256], dt.float32r)
    gw = din("gw", [E // GK, 128, GK, 2, 512], dt.bfloat16)
    corrv = din("corrv", [AP_, 512], dt.bfloat16)
    cb1c = din("cb1c", [128, 4], dt.float32)
    w2t = din("w2t", [128, 4, 256], dt.float32r)
    b2c = din("b2c", [128, 2], dt.float32)
    mw = din("mw", [128, NH, 2, 256], dt.float32r)
    vw = din("vw", [128, NH, 2, 256], dt.float32r)
    fpw = din("fpw", [128, 2, NH, 2, 256], dt.float16)
    fpbr = din("fpbr", [1, 2, NH, 2, 128], dt.float32r)
    lngb = din("lngb", [128, 2, 2, NH], dt.float32)
    connc = din("connc", [A, A], dt.float32)
    outwtb = din("outwtb", [128, 2, OUT], dt.float16)
    onesmat = din("onesmat", [128, 128], dt.float32r)
    lngr = din("lngr", [1, NH, 2, 128], dt.float32r)

    out = nc.dram_tensor("out", [BL, A, OUT], dt.float32, kind="ExternalOutput")

    with tile.TileContext(nc) as tc:
        nc.gpsimd.load_library(library_config.local_scatter)

        # Chain ACT ops in emission order so the scheduler cannot interleave
        # activations from different pwp table sets (each switch costs 1.3us).
        _last_act = [None]

        def act(*args, **kw):
            return nc.scalar.activation(*args, **kw)

        with tc.tile_pool(name="const", bufs=1) as cpool:
            sliota_t = cpool.tile([AP_, AP_], dt.float32, tag="sliota")
            nc.scalar.dma_start(sliota_t[:], sliota.ap())
            dupB4_t = cpool.tile([4, 32], dt.float32r, tag="dupB4")
            nc.scalar.dma_start(dupB4_t[:], dupB4.ap())
            addp_t = cpool.tile([32, 1], dt.float32, tag="addp")
            nc.scalar.dma_start(addp_t[:], addp.ap())
            sbp_t = cpool.tile([32, 1], dt.float32, tag="sbp")
            nc.scalar.dma_start(sbp_t[:], sbp.ap())
            w1pos_t = cpool.tile([34, 256], dt.float32r, tag="w1pos")
            nc.scalar.dma_start(w1pos_t[:], w1pos.ap())
            corrv_t = cpool.tile([AP_, 512], dt.bfloat16, tag="corrv")
            nc.scalar.dma_start(corrv_t[:], corrv.ap())
            cb1c_t = cpool.tile([128, 4], dt.float32, tag="cb1c")
            nc.scalar.dma_start(cb1c_t[:], cb1c.ap())
            w2t_t = cpool.tile([128, 4, 256], dt.float32r, tag="w2t")
            nc.scalar.dma_start(w2t_t[:], w2t.ap())
            b2c_t = cpool.tile([128, 2], dt.float32, tag="b2c")
            nc.scalar.dma_start(b2c_t[:], b2c.ap())
            cap_t = cpool.tile([AP_, 1], dt.float32, tag="cap")
            nc.vector.memset(cap_t[:], 255.0)
            eps_t = cpool.tile([128, 1], dt.float32, tag="eps")
            nc.vector.memset(eps_t[:], 1e-5)
            cnt_t = cpool.tile([AP_, BL], dt.float32, tag="cnt")
            # transposed entry arrays ([slot, b, pair]), persist into phase B
            # rcT packs row*256 (rc=0) and col (rc=1) for a single rcflat DMA
            rcT = cpool.tile([AP_, 2, NPAIR], dt.float32r, tag="rcT")
            valTall = cpool.tile([AP_, NPAIR], dt.float32r, tag="valT")
            invTall = cpool.tile([AP_, NPAIR], dt.bfloat16, tag="invT")
            # h ping-pong tiles (persist across phases)
            htiles = [cpool.tile([128, 2, NPAIR], dt.float32r,
                                 name=f"hst{i}", tag=f"hst{i}")
                      for i in range(2)]

            # ------------- Phase A: scan + scatter + transpose, per b -------
            with tc.tile_pool(name="pA", bufs=2) as pA, \
                 tc.tile_pool(name="pA1", bufs=1) as pA1, \
                 tc.tile_pool(name="psA", bufs=2,
                              space=bass.MemorySpace.PSUM) as psA:
                identf_t = pA1.tile([128, 128], dt.float32r, tag="identf")
                nc.scalar.dma_start(identf_t[:], identf.ap())
                identb_t = pA1.tile([128, 128], dt.bfloat16, tag="identb")
                nc.scalar.dma_start(identb_t[:], identb.ap())
                ptab_t = pA1.tile([AP_, D], dt.uint16, tag="ptab")
                nc.gpsimd.dma_start(ptab_t[:],
                                    ptab.ap().broadcast_to([AP_, D]))
                DC = 2500
                NJ = D // DC
                for b in range(BL):
                    dvps = []
                    dpps = []
                    ct_prev = None
                    for j in range(NJ):
                        dsl = slice(j * DC, (j + 1) * DC)
                        xt = pA.tile([AP_, DC], dt.bfloat16, tag="xt", bufs=3,
                                     name="xt")
                        nc.sync.dma_start(xt[:], xb.ap()[b, :, dsl])
                        mk = pA.tile([AP_, DC], dt.bfloat16, tag="mk", bufs=3,
                                     name="mk")
                        nc.vector.tensor_scalar(mk[:], xt[:], 0.0, None,
                                                Alu.not_equal)
                        ct = pA.tile([AP_, DC], dt.bfloat16, tag="ct", bufs=3,
                                     name="ct")
                        nc.vector.tensor_tensor_scan(
                            ct[:], mk[:], cap_t[:].broadcast_to([AP_, DC]),
                            0.0 if j == 0 else ct_prev[:, DC - 1:DC],
                            Alu.add, Alu.min)
                        ct_prev = ct
                        # sg = 300*mk - 301 (ACT keeps DVE free for the scan)
                        sg = pA.tile([AP_, DC], dt.bfloat16, tag="sg", bufs=3,
                                     name="sg")
                        act(sg[:], mk[:], Act.Copy, scale=300.0, bias=-301.0)
                        # idx = ct + sg  (= ct-1 for valid, out-of-range else)
                        idx16 = pA.tile([AP_, DC], dt.int16, tag="idx", bufs=3,
                                        name="idx16")
                        nc.vector.tensor_tensor(idx16[:], ct[:], sg[:],
                                                Alu.add)
                        dvp = pA.tile([AP_, 256], dt.bfloat16, tag=f"dvp{j}",
                                      name=f"dvp{j}")
                        nc.gpsimd.local_scatter(dvp[:], xt[:], idx16[:],
                                                channels=AP_, num_elems=256,
                                                num_idxs=DC)
                        dpp = pA.tile([AP_, 256], dt.uint16, tag=f"dpp{j}",
                                      name=f"dpp{j}")
                        nc.gpsimd.local_scatter(dpp[:], ptab_t[:, dsl],
                                                idx16[:], channels=AP_,
                                                num_elems=256, num_idxs=DC)
                        dvps.append(dvp)
                        dpps.append(dpp)
                    nc.vector.tensor_copy(cnt_t[:, b:b + 1],
                                          ct_prev[:, DC - 1:DC])
                    # merge chunk scatters (disjoint support)
                    va = pA.tile([AP_, 256], dt.bfloat16, tag="va", name="va")
                    vb = pA.tile([AP_, 256], dt.bfloat16, tag="vb", name="vb")
                    nc.vector.tensor_tensor(va[:], dvps[0][:], dvps[1][:],
                                            Alu.add)
                    nc.vector.tensor_tensor(vb[:], dvps[2][:], dvps[3][:],
                                            Alu.add)
                    dval = pA.tile([AP_, 256], dt.bfloat16, tag="dval",
                                   name="dval")
                    nc.vector.tensor_tensor(dval[:], va[:], vb[:], Alu.add)
                    pa = pA.tile([AP_, 256], dt.float32, tag="pa", name="pa")
                    pb_ = pA.tile([AP_, 256], dt.float32, tag="pb", name="pb")
                    nc.vector.tensor_tensor(pa[:], dpps[0][:], dpps[1][:],
                                            Alu.add)
                    nc.vector.tensor_tensor(pb_[:], dpps[2][:], dpps[3][:],
                                            Alu.add)
                    packf = pA.tile([AP_, 256], dt.float32, tag="packf",
                                    name="packf")
                    nc.vector.tensor_tensor(packf[:], pa[:], pb_[:], Alu.add)

                    # invalid-slot grid: (slot >= count)
                    invg = pA.tile([AP_, AP_], dt.bfloat16, tag="invg",
                                   name="invg")
                    nc.vector.tensor_scalar(invg[:], sliota_t[:],
                                            cnt_t[:, b:b + 1], None, Alu.is_ge)
                    # coords decode (DVE magic round): row = rnd(packf/256-.5)
                    rowt = pA.tile([AP_, AP_], dt.float32, tag="rowt",
                                   name="rowt")
                    nc.vector.tensor_scalar(rowt[:], packf[:, :AP_],
                                            2.0 ** -8, -0.498046875,
                                            Alu.mult, Alu.add)
                    rowf = pA.tile([AP_, AP_], dt.float32r, tag="rowf",
                                   name="rowf")
                    nc.vector.tensor_scalar(rowf[:], rowt[:], MAGIC, -MAGIC,
                                            Alu.add, Alu.add)
                    colf = pA.tile([AP_, AP_], dt.float32r, tag="colf",
                                   name="colf")
                    nc.vector.scalar_tensor_tensor(colf[:], rowf[:], -256.0,
                                                   packf[:, :AP_], Alu.mult,
                                                   Alu.add)

                    # transposes -> [slot, pair] layout
                    tv = psA.tile([AP_, AP_], dt.bfloat16, tag="tv", name="tv")
                    nc.tensor.transpose(tv[:], dval[:, :AP_],
                                        identb_t[:AP_, :AP_])
                    nc.vector.tensor_copy(valTall[:, b * A:(b + 1) * A],
                                          tv[:, :A])
                    tr_ = psA.tile([AP_, AP_], dt.float32r, tag="tr",
                                   name="tr_")
                    nc.tensor.transpose(tr_[:], rowf[:], identf_t[:AP_, :AP_])
                    nc.vector.tensor_copy(rcT[:, 0, b * A:(b + 1) * A],
                                          tr_[:, :A])
                    tcl = psA.tile([AP_, AP_], dt.float32r, tag="tc",
                                   name="tcl")
                    nc.tensor.transpose(tcl[:], colf[:], identf_t[:AP_, :AP_])
                    nc.vector.tensor_copy(rcT[:, 1, b * A:(b + 1) * A],
                                          tcl[:, :A])
                    ti = psA.tile([AP_, AP_], dt.bfloat16, tag="ti", name="ti")
                    nc.tensor.transpose(ti[:], invg[:], identb_t[:AP_, :AP_])
                    nc.vector.tensor_copy(invTall[:, b * A:(b + 1) * A],
                                          ti[:, :A])

            # phase C weights: loaded once phase A's queue traffic is done
            onesmat_t = cpool.tile([128, 128], dt.float32r, tag="onesmat")
            nc.scalar.dma_start(onesmat_t[:], onesmat.ap())
            mw_t = cpool.tile([128, NH, 2, 256], dt.float32r, tag="mw")
            nc.scalar.dma_start(mw_t[:], mw.ap())
            vw_t = cpool.tile([128, NH, 2, 256], dt.float32r, tag="vw")
            nc.scalar.dma_start(vw_t[:], vw.ap())
            fpw_t = cpool.tile([128, 2, NH, 2, 256], dt.float16, tag="fpw")
            nc.scalar.dma_start(fpw_t[:], fpw.ap())
            fpbr_t = cpool.tile([1, 2, NH, 2, 128], dt.float32r, tag="fpbr")
            nc.scalar.dma_start(fpbr_t[:], fpbr.ap())
            ones4_t = cpool.tile([1, 512], dt.float32r, tag="ones4")
            nc.scalar.dma_start(ones4_t[:], onesrow.ap()[0:1, 0:512])
            lngb_t = cpool.tile([128, 2, 2, NH], dt.float32, tag="lngb")
            nc.scalar.dma_start(lngb_t[:], lngb.ap())
            connc_t = cpool.tile([A, A], dt.float32, tag="connc")
            nc.scalar.dma_start(connc_t[:], connc.ap())
            lngr_t = cpool.tile([1, NH, 2, 128], dt.float32r, tag="lngr")
            nc.scalar.dma_start(lngr_t[:], lngr.ap())

            # ------------- Phase B: entry encoder + compression -------------
            with tc.tile_pool(name="pB", bufs=3) as pB, \
                 tc.tile_pool(name="psH", bufs=1,
                              space=bass.MemorySpace.PSUM) as psH, \
                 tc.tile_pool(name="psZ", bufs=2,
                              space=bass.MemorySpace.PSUM) as psZ, \
                 tc.tile_pool(name="psP", bufs=2,
                              space=bass.MemorySpace.PSUM) as psP:
                h1ps = [psH.tile([128, NPAIR], dt.float32,
                                 name=f"h1_{mq}", tag=f"h1_{mq}")
                        for mq in range(4)]
                first = True

                chunks = {}
                rcflats = {}
                ntrCs = {}

                def rcflat_dma(c):
                    if c >= NB:
                        return
                    k0 = c * NSLOT_CHUNK
                    ksl = slice(k0, k0 + NSLOT_CHUNK)
                    rcflat = pB.tile([4, NSLOT_CHUNK, NPAIR], dt.float32r,
                                     tag="rcflat", bufs=4, name="rcflat")
                    for rr in range(4):
                        nc.sync.dma_start(rcflat[rr:rr + 1],
                                          rcT[ksl, rr % 2])
                    rcflats[c] = rcflat

                def featc_dma(c):
                    if c >= NB:
                        return
                    k0 = c * NSLOT_CHUNK
                    ksl = slice(k0, k0 + NSLOT_CHUNK)
                    featC = pB.tile([34, NSLOT_CHUNK, NPAIR], dt.float32r,
                                    tag="featC", name="featC")
                    nc.sync.dma_start(featC[32:33], valTall[ksl])
                    nc.sync.dma_start(featC[33:34], onesrow.ap())
                    chunks[c] = featC

                def proj_mm(c):
                    k0 = c * NSLOT_CHUNK
                    rcflat = rcflats.pop(c)
                    ntrC = pB.tile([32, NSLOT_CHUNK, NPAIR], dt.float16,
                                   tag="ntrC", bufs=2, name="ntrC")
                    nslot = min(NSLOT_CHUNK, E - k0)
                    for ks in range(nslot):
                        proj = psP.tile([32, NPAIR], dt.float32,
                                        tag="proj", name="proj")
                        nc.tensor.matmul(proj[:], dupB4_t[:], rcflat[:, ks, :],
                                         start=True, stop=True)
                        # magic round on DVE: u2 = rnd(proj+shift) + MAGIC
                        u2 = pB.tile([32, NPAIR], dt.float32, tag="u2",
                                     bufs=1, name="u2")
                        nc.vector.tensor_scalar(u2[:], proj[:], addp_t[:],
                                                MAGIC, Alu.add, Alu.add)
                        # ntr = rnd(proj+shift) - proj
                        nc.vector.scalar_tensor_tensor(
                            ntrC[:, ks, :], u2[:], -MAGIC, proj[:],
                            Alu.add, Alu.subtract)
                    ntrCs[c] = ntrC

                def proj_sin(c, half):
                    # sin(-2pi*ntr + 2pi*shift) = sin(2pi*(x+shift)); emitted
                    # in halves interleaved between zcomp silus so the ACT
                    # queue reaches them before the chunk boundary
                    k0 = c * NSLOT_CHUNK
                    nslot = min(NSLOT_CHUNK, E - k0)
                    h = (nslot + 1) // 2
                    lo, hi = (0, h) if half == 0 else (h, nslot)
                    if lo >= hi:
                        return
                    ntrC = ntrCs[c]
                    featC = chunks[c]
                    act(featC[:32, lo:hi, :], ntrC[:, lo:hi, :], Act.Sin,
                        scale=-TWO_PI, bias=sbp_t[:])

                def zcomp(c, mid=None):
                    """zpass for chunk c+1 interleaved with comppass for c."""
                    nonlocal first
                    nz = min(NSLOT_CHUNK, E - (c + 1) * NSLOT_CHUNK) \
                        if c + 1 < NB else 0
                    ncp = min(NSLOT_CHUNK, E - c * NSLOT_CHUNK) if c >= 0 else 0
                    sts_new = []
                    for ks in range(NSLOT_CHUNK):
                        if mid is not None and ks in (3, 6):
                            mid(0 if ks == 3 else 1)
                        if ks < nz:
                            featC = chunks[c + 1]
                            s_t = pB.tile([128, 2, NPAIR], dt.bfloat16,
                                          tag="s", bufs=2 * NSLOT_CHUNK,
                                          name="s")
                            for mh in range(2):
                                zb = psZ.tile([128, 512], dt.float32,
                                              tag="zb", name="zb")
                                nc.tensor.matmul(
                                    zb[:, :NPAIR],
                                    w1pos_t[:, mh * 128:(mh + 1) * 128],
                                    featC[:, ks, :], start=True, stop=True)
                                act(s_t[:, mh, :], zb[:, :NPAIR], Act.Silu)
                            sts_new.append(s_t)
                        # comppass for chunk c, slot ks
                        if ks < ncp:
                            k = c * NSLOT_CHUNK + ks
                            if k % GK == 0:
                                zcomp.gk4 = pB.tile([128, GK, 2, 512],
                                                    dt.bfloat16, tag="gk",
                                                    bufs=2, name="gk")
                                nc.scalar.dma_start(zcomp.gk4[:],
                                                    gw.ap()[k // GK])
                            gk4 = zcomp.gk4
                            s_t = zcomp.sts[ks]
                            for kh in range(2):
                                for mq in range(4):
                                    nc.tensor.matmul(
                                        h1ps[mq][:],
                                        gk4[:, k % GK, kh,
                                            mq * 128:(mq + 1) * 128],
                                        s_t[:, kh, :],
                                        start=first, stop=False,
                                        skip_group_check=True)
                                first = False
                    zcomp.sts = sts_new

                NB = (E + NSLOT_CHUNK - 1) // NSLOT_CHUNK
                for cc in range(4):
                    rcflat_dma(cc)
                featc_dma(0)
                featc_dma(1)
                proj_mm(0)
                proj_sin(0, 0)
                proj_sin(0, 1)
                featc_dma(2)
                zcomp(-1, mid=lambda half: (proj_mm(1) if half == 0 else None,
                                            proj_sin(1, half))
                      )     # prime: zpass chunk 0; chunk-1 skeleton mid-loop
                for c in range(NB):
                    rcflat_dma(c + 4)
                    featc_dma(c + 3)
                    if c + 2 < NB:
                        proj_mm(c + 2)
                    # comp chunk c + zpass chunk c+1; sin halves for c+2
                    # interleave mid-loop so ACT reaches them early
                    zcomp(c, mid=(lambda half, cc=c:
                                  proj_sin(cc + 2, half))
                          if c + 2 < NB else None)

                # pad-token corrections + comp layer 2
                for mq in range(4):
                    nc.tensor.matmul(h1ps[mq][:],
                                     corrv_t[:, mq * 128:(mq + 1) * 128],
                                     invTall[:], start=False, stop=True,
                                     skip_group_check=True)
                hsw = pB.tile([128, 4, NPAIR], dt.float32r, tag="hsw",
                              bufs=1)
                for mq in range(4):
                    act(hsw[:, mq, :], h1ps[mq][:], Act.Silu,
                        bias=cb1c_t[:, mq:mq + 1])
                for ih in range(2):
                    zb2 = psZ.tile([128, 512], dt.float32, tag="zb",
                                   name="zb2")
                    for kq in range(4):
                        nc.tensor.matmul(zb2[:, :NPAIR],
                                         w2t_t[:, kq, ih * 128:(ih + 1) * 128],
                                         hsw[:, kq, :],
                                         start=(kq == 0), stop=(kq == 3))
                    act(htiles[0][:, ih, :], zb2[:, :NPAIR], Act.Identity,
                        bias=b2c_t[:, ih:ih + 1])

            # ------------- Phase C: GAT message passing ---------------------
            with tc.tile_pool(name="pD0", bufs=1) as pD0:
                # bf16 out_W, prefetched in chunks between attention steps
                woutb = pD0.tile([128, 2, OUT], dt.float16, tag="woutb")
                WCH = OUT // 4

                def wout_prefetch(i):
                    sl = slice(i * WCH, (i + 1) * WCH)
                    nc.sync.dma_start(woutb[:, :, sl], outwtb.ap()[:, :, sl])

                phaseC = tc.tile_pool(name="pC1", bufs=1)
                pC1 = phaseC.__enter__()
                pC2 = tc.tile_pool(name="pC2", bufs=3).__enter__()
                psQ = tc.tile_pool(name="psQ", bufs=3,
                                   space=bass.MemorySpace.PSUM).__enter__()
                psS = tc.tile_pool(name="psS", bufs=2,
                                   space=bass.MemorySpace.PSUM).__enter__()

                hcur = htiles[0]
                for step in range(STEPS):
                    wout_prefetch(step)
                    # bf16 copy of h shared by all heads (scores rhs)
                    hb = pC2.tile([128, 2, NPAIR], dt.bfloat16, tag="hb",
                                  name="hb")
                    for kh in range(2):
                        nc.vector.tensor_copy(hb[:, kh, :], hcur[:, kh, :])
                    Rbs, VTs, aTs = [], [], []
                    # --- R = (qW^T kW / 16)^T-projected h;  V ---
                    for n in range(NH):
                        rp = psQ.tile([128, 2, 512], dt.float32, tag="q2",
                                      name="rp")
                        for jh in range(2):
                            for kh in range(2):
                                nc.tensor.matmul(
                                    rp[:, jh, :NPAIR],
                                    mw_t[:, n, kh, jh * 128:(jh + 1) * 128],
                                    hcur[:, kh, :],
                                    start=(kh == 0), stop=(kh == 1))
                        rb = pC1.tile([128, 2, NPAIR], dt.bfloat16,
                                      tag=f"rb{n}", name=f"rb{n}")
                        nc.vector.tensor_copy(rb[:], rp[:, :, :NPAIR])
                        Rbs.append(rb)
                        vp = psQ.tile([128, 2, 512], dt.float32, tag="q2",
                                      name="vp")
                        for b in range(BL):
                            for kh in range(2):
                                nc.tensor.matmul(
                                    vp[:A, b // 2, (b % 2) * 256:
                                       (b % 2) * 256 + 256],
                                    hcur[:, kh, b * A:(b + 1) * A],
                                    vw_t[:, n, kh, :],
                                    start=(kh == 0), stop=(kh == 1))
                        vt_t = pC1.tile([AP_, BL, 256], dt.bfloat16,
                                        tag=f"vts{n}", name=f"vts{n}")
                        nc.vector.tensor_copy(
                            vt_t[:A].reshape([A, 2, 2, 256]),
                            vp[:A].reshape([A, 2, 2, 256]))
                        VTs.append(vt_t)
                    # --- scores + softmax, batched over b per head ---
                    for n in range(NH):
                        scp = psS.tile([A, 512], dt.float32, tag="sc",
                                       name="scp")
                        for b in range(BL):
                            for kh in range(2):
                                nc.tensor.matmul(
                                    scp[:, b * 128:b * 128 + A],
                                    Rbs[n][:, kh, b * A:(b + 1) * A],
                                    hb[:, kh, b * A:(b + 1) * A],
                                    start=(kh == 0), stop=(kh == 1))
                        sca = pC2.tile([A, BL, A], dt.float32, tag="sca",
                                       name="sca")
                        nc.vector.tensor_tensor(
                            sca[:],
                            scp[:].reshape([A, BL, 128])[:, :, :A],
                            connc_t[:].reshape([A, 1, A])
                            .broadcast_to([A, BL, A]),
                            Alu.add)
                        esc = pC2.tile([A, BL, A], dt.float32, tag="esc",
                                       name="esc")
                        act(esc[:], sca[:], Act.Exp)
                        sm = pC2.tile([A, BL], dt.float32, tag="sm", name="sm")
                        nc.vector.tensor_reduce(sm[:], esc[:], Ax.X, Alu.add)
                        rs = pC2.tile([A, BL], dt.float32, tag="rs", name="rs")
                        nc.vector.reciprocal(rs[:], sm[:])
                        alp = pC2.tile([128, BL, 128], dt.bfloat16, tag="alp",
                                       name="alp")
                        nc.vector.tensor_tensor(
                            alp[:A, :, :A], esc[:],
                            rs[:].reshape([A, BL, 1]).broadcast_to([A, BL, A]),
                            Alu.mult)
                        at_t = pC1.tile([128, BL, 128], dt.bfloat16,
                                        tag=f"ats{n}", name=f"ats{n}")
                        for b in range(BL):
                            nc.sync.dma_start_transpose(at_t[:, b, :],
                                                        alp[:, b, :])
                        aTs.append(at_t)
                    # --- message + MLP + LN stats ---
                    ms, vvs, tss = [], [], []
                    for n in range(NH):
                        hmp = psQ.tile([128, 2, 512], dt.float32, tag="q2",
                                       name="hmp")
                        for jh in range(2):
                            for b in range(BL):
                                nc.tensor.matmul(
                                    hmp[:, jh, b * A:(b + 1) * A],
                                    VTs[n][:A, b, jh * 128:(jh + 1) * 128],
                                    aTs[n][:A, b, :A],
                                    start=True, stop=True,
                                    skip_group_check=True)
                        hs_t = pC2.tile([128, 2, NPAIR], dt.bfloat16,
                                        tag="hs")
                        act(hs_t[:], hmp[:, :, :NPAIR], Act.Silu)
                        t1p = psQ.tile([128, 2, 512], dt.float32, tag="q2",
                                       name="t1p")
                        for ih in range(2):
                            for jh in range(2):
                                nc.tensor.matmul(
                                    t1p[:, ih, :NPAIR],
                                    fpw_t[:, 0, n, jh, ih * 128:(ih + 1) * 128],
                                    hs_t[:, jh, :],
                                    start=(jh == 0), stop=(jh == 1))
                        t1s = pC2.tile([128, 2, NPAIR], dt.bfloat16, tag="t1s")
                        for ih in range(2):
                            act(t1s[:, ih, :], t1p[:, ih, :NPAIR], Act.Silu,
                                bias=fpb_t[:, 0, ih, n:n + 1])
                        t2p = psQ.tile([128, 2, 512], dt.float32, tag="q2",
                                       name="t2p")
                        for ih in range(2):
                            for jh in range(2):
                                nc.tensor.matmul(
                                    t2p[:, ih, :NPAIR],
                                    fpw_t[:, 1, n, jh, ih * 128:(ih + 1) * 128],
                                    t1s[:, jh, :],
                                    start=(jh == 0), stop=(jh == 1))
                        ts_t = pC1.tile([128, 2, NPAIR], dt.float32r,
                                        tag=f"ts{n}")
                        for ih in range(2):
                            act(ts_t[:, ih, :], t2p[:, ih, :NPAIR],
                                Act.Identity, bias=fpb_t[:, 1, ih, n:n + 1])
                        tsq = pC2.tile([128, 2, NPAIR], dt.float32r, tag="tsq")
                        nc.vector.tensor_tensor(tsq[:], ts_t[:], ts_t[:],
                                                Alu.mult)
                        mtp = psS.tile([A, 512], dt.float32, tag="sc",
                                       name="mtp")
                        for ih in range(2):
                            nc.tensor.matmul(mtp[:1, :NPAIR],
                                             onesmat_t[:, 0:1],
                                             ts_t[:, ih, :],
                                             start=(ih == 0), stop=(ih == 1))
                        vtp = psS.tile([A, 512], dt.float32, tag="sc",
                                       name="vtp")
                        for ih in range(2):
                            nc.tensor.matmul(vtp[:1, :NPAIR],
                                             onesmat_t[:, 0:1],
                                             tsq[:, ih, :],
                                             start=(ih == 0), stop=(ih == 1))
                        m_t = pC1.tile([1, NPAIR], dt.float32r, tag=f"m{n}")
                        act(m_t[:], mtp[:1, :NPAIR], Act.Identity,
                            scale=1.0 / 256.0)
                        msq = pC2.tile([1, NPAIR], dt.float32r, tag="msq")
                        nc.vector.tensor_tensor(msq[:], m_t[:], m_t[:],
                                                Alu.mult)
                        vv = pC1.tile([1, NPAIR], dt.float32, tag=f"vv{n}")
                        nc.vector.scalar_tensor_tensor(
                            vv[:], vtp[:1, :NPAIR], 1.0 / 256.0, msq[:],
                            Alu.mult, Alu.subtract)
                        ms.append(m_t); vvs.append(vv); tss.append(ts_t)
                    # --- rstd = exp(-0.5*ln(v+eps)) (one ln+exp table set) ---
                    rstds = []
                    lnvs = []
                    for n in range(NH):
                        lnv = pC1.tile([1, NPAIR], dt.float32, tag=f"lnv{n}",
                                       name=f"lnv{n}")
                        act(lnv[:], vvs[n][:], Act.Ln, bias=eps_t[:1])
                        lnvs.append(lnv)
                    for n in range(NH):
                        rstd = pC1.tile([1, NPAIR], dt.float32r,
                                        tag=f"rsd{n}", name=f"rsd{n}")
                        act(rstd[:], lnvs[n][:], Act.Exp, scale=-0.5)
                        rstds.append(rstd)
                    # --- hnew = sum_n ts_n*(g*rstd)_n + bsum - sum_n(g*m*rstd)
                    hnew = htiles[(step + 1) % 2]
                    mrs = []
                    for n in range(NH):
                        mr = pC1.tile([1, NPAIR], dt.float32r, tag=f"mr{n}",
                                      name=f"mr{n}")
                        nc.vector.tensor_tensor(mr[:], ms[n][:], rstds[n][:],
                                                Alu.mult)
                        mrs.append(mr)
                    for ih in range(2):
                        mgp = psQ.tile([128, 2, 512], dt.float32, tag="q2",
                                       name="mgp")
                        for n in range(NH):
                            nc.tensor.matmul(mgp[:, 0, :NPAIR],
                                             lngr_t[0:1, n, ih, :],
                                             mrs[n][:], start=(n == 0),
                                             stop=(n == 3))
                        for n in range(NH):
                            nc.tensor.matmul(mgp[:, 1, :NPAIR],
                                             lngr_t[0:1, n, ih, :],
                                             rstds[n][:], start=(n == 0),
                                             stop=(n == 3),
                                             skip_group_check=True)
                            if n == 0:
                                nc.vector.tensor_tensor(hnew[:, ih, :],
                                                        tss[n][:, ih, :],
                                                        mgp[:, 1, :NPAIR],
                                                        Alu.mult)
                            else:
                                u1 = pC2.tile([128, NPAIR], dt.float32,
                                              tag="u1", name="u1")
                                nc.vector.tensor_tensor(u1[:],
                                                        tss[n][:, ih, :],
                                                        mgp[:, 1, :NPAIR],
                                                        Alu.mult)
                                nc.vector.tensor_tensor(hnew[:, ih, :],
                                                        hnew[:, ih, :], u1[:],
                                                        Alu.add)
                        nc.vector.scalar_tensor_tensor(
                            hnew[:, ih, :], hnew[:, ih, :],
                            lngb_t[:, 1, ih, 0:1], mgp[:, 0, :NPAIR],
                            Alu.add, Alu.subtract)
                    hcur = hnew
                wout_prefetch(3)

                # ------------- Phase D: output projection -------------------
                with tc.tile_pool(name="pD", bufs=3) as pD, \
                     tc.tile_pool(name="psD", bufs=2,
                                  space=bass.MemorySpace.PSUM) as psD:
                    hfb = pD.tile([128, 2, NPAIR], dt.float16, tag="hfb",
                                  bufs=1)
                    for ih in range(2):
                        nc.vector.tensor_copy(hfb[:, ih, :], hcur[:, ih, :])
                    for ci, c0 in enumerate(range(0, OUT, OUTC)):
                        w = min(OUTC, OUT - c0)
                        pop = psD.tile([A, BL, OUTC], dt.float32, tag="pop")
                        for b in range(BL):
                            for ih in range(2):
                                nc.tensor.matmul(pop[:, b, :w],
                                                 hfb[:, ih, b * A:(b + 1) * A],
                                                 woutb[:, ih, c0:c0 + w],
                                                 start=(ih == 0),
                                                 stop=(ih == 1),
                                                 skip_group_check=True)
                        ost = pD.tile([A, BL, OUTC], dt.float32, tag="ost")
                        if ci % 2 == 0:
                            act(ost[:], pop[:], Act.Copy)
                        else:
                            nc.vector.tensor_copy(ost[:], pop[:])
                        nc.gpsimd.dma_start(
                            out.ap()[:, :, c0:c0 + w]
                            .rearrange("b a c -> a b c"),
                            ost[:, :, :w])

    nc.compile()
    return nc


def host_prep(inputs):
    f32 = np.float32
    x = np.asarray(inputs["x"], f32)
    enc_W1 = np.asarray(inputs["enc_W1"], f32)
    enc_b1 = np.asarray(inputs["enc_b1"], f32)
    enc_W2 = np.asarray(inputs["enc_W2"], f32)
    enc_b2 = np.asarray(inputs["enc_b2"], f32)
    comp_W1 = np.asarray(inputs["comp_W1"], f32)
    comp_b1 = np.asarray(inputs["comp_b1"], f32)
    comp_W2 = np.asarray(inputs["comp_W2"], f32)
    comp_b2 = np.asarray(inputs["comp_b2"], f32)
    pad = np.asarray(inputs["pad_token"], f32)
    fB = np.asarray(inputs["fourier_B"], f32)
    qW = np.asarray(inputs["qW"], f32)
    kW = np.asarray(inputs["kW"], f32)
    vW = np.asarray(inputs["vW"], f32)
    fp_W1 = np.asarray(inputs["fp_W1"], f32)
    fp_b1 = np.asarray(inputs["fp_b1"], f32)
    fp_W2 = np.asarray(inputs["fp_W2"], f32)
    fp_b2 = np.asarray(inputs["fp_b2"], f32)
    ln_g = np.asarray(inputs["ln_g"], f32)
    ln_b = np.asarray(inputs["ln_b"], f32)
    conn = np.asarray(inputs["connectivity"], f32)
    out_W = np.asarray(inputs["out_W"], f32)

    M = comp_W1.reshape(512, E, HID)
    G = np.einsum('rkj,jl->rkl', M, enc_W2, optimize=True)      # [512, E, 256]
    feat0 = np.concatenate([[0.0], np.zeros(16, f32),
                            np.ones(16, f32)]).astype(f32)
    z00 = feat0 @ enc_W1.T + enc_b1
    e00 = (z00 / (1 + np.exp(-z00))) @ enc_W2.T + enc_b2
    corrV = np.einsum('rkj,j->rk', M, (pad - e00))               # [512, E]
    cb1p = comp_b1 + np.einsum('rkj,j->r', M, enc_b2)

    # gw[g, p, ki, kh, r] = G[r, g*GK+ki, kh*128+p]
    Gr = G.reshape(512, E // GK, GK, 2, 128)          # [r, g, ki, kh, p]
    gw = np.ascontiguousarray(Gr.transpose(1, 4, 2, 3, 0)).astype(bf16)

    corrv = np.zeros((AP_, 512), f32)
    corrv[:E] = corrV.T
    corrv = corrv.astype(bf16)

    # split fourier coefs: 10-bit-quantized high part (integer-exact products
    # through the f32r matmul) + small residual; row features carry row*256
    # so the row coefficients are pre-divided by 256 (exact in fp32)
    bhi = np.round(fB * 1024.0) / 1024.0
    blo = (fB - bhi).astype(f32)
    bhi = bhi.astype(f32)
    dupB4 = np.zeros((4, 32), f32)
    dupB4[0, :16] = bhi[:, 0]; dupB4[0, 16:] = bhi[:, 0]
    dupB4[1, :16] = bhi[:, 1]; dupB4[1, 16:] = bhi[:, 1]
    dupB4[2, :16] = blo[:, 0]; dupB4[2, 16:] = blo[:, 0]
    dupB4[3, :16] = blo[:, 1]; dupB4[3, 16:] = blo[:, 1]
    # per-partition phase shift (+0.25 on the cos half), applied before the
    # magic-round add so it survives fp32; matching sin-arg bias
    addp = np.zeros((32, 1), f32)
    addp[16:] = 0.25
    sbp = np.zeros((32, 1), f32)
    sbp[16:] = TWO_PI * 0.25

    w1pos = np.zeros((34, 256), f32)
    w1pos[:32] = enc_W1[:, 1:33].T
    w1pos[32] = enc_W1[:, 0]
    w1pos[33] = enc_b1

    cb1c = np.ascontiguousarray(cb1p.reshape(4, 128).T)
    w2t = np.ascontiguousarray(
        comp_W2.T.reshape(4, 128, 256).transpose(1, 0, 2))
    b2c = np.ascontiguousarray(comp_b2.reshape(2, 128).T)

    # fold qW^T kW (and the 1/sqrt(HID) score scale) into one matrix
    Mn = np.einsum('nji,njk->nik', qW * 0.25, kW * 0.25)   # [NH, 256, 256]
    mw = np.ascontiguousarray(
        Mn.reshape(NH, 2, 128, 256).transpose(2, 0, 1, 3))  # [128,NH,2,256]
    vw = np.ascontiguousarray(
        vW.transpose(0, 2, 1).reshape(NH, 2, 128, 256)
        .transpose(2, 0, 1, 3))                             # [128,NH,2,256]
    fpw = np.stack([fp_W1, fp_W2])                    # [2, n, i, j]
    fpw = fpw.transpose(0, 1, 3, 2).reshape(2, NH, 2, 128, 256)
    fpw = np.ascontiguousarray(fpw.transpose(3, 0, 1, 2, 4)).astype(np.float16)
    fpbr = np.ascontiguousarray(
        np.stack([fp_b1, fp_b2]).reshape(2, NH, 2, 128))[None]
    lngb = np.zeros((128, 2, 2, NH), f32)
    lg = (ln_g / 4.0).reshape(NH, 2, 128)             # [n, ih, p]
    lngb[:, 0, :, :] = lg.transpose(2, 1, 0)
    bsum = (ln_b / 4.0).sum(0).reshape(2, 128)        # [ih, p]
    lngb[:, 1, :, 0] = bsum.T

    outwtb = np.ascontiguousarray(
        out_W.T.reshape(2, 128, OUT).transpose(1, 0, 2)).astype(np.float16)

    ptab = (np.arange(D, dtype=np.uint32) // NGRID * 256
            + np.arange(D, dtype=np.uint32) % NGRID).astype(np.uint16)
    sliota = np.ascontiguousarray(
        np.broadcast_to(np.arange(AP_, dtype=f32)[None, :], (AP_, AP_)))
    identf = np.eye(128, dtype=f32)
    identb = np.eye(128, dtype=f32).astype(bf16)

    shared = {
        "ptab": ptab[None, :], "sliota": sliota, "identf": identf,
        "identb": identb, "onesrow": np.ones((1, ENT_C), f32),
        "dupB4": dupB4, "addp": addp, "sbp": sbp, "w1pos": w1pos, "gw": gw,
        "corrv": corrv, "cb1c": cb1c, "w2t": w2t, "b2c": b2c, "mw": mw,
        "vw": vw, "fpw": fpw, "fpbr": fpbr, "lngb": lngb,
        "connc": np.ascontiguousarray(conn), "outwtb": outwtb,
        "onesmat": np.ones((128, 128), f32),
        "lngr": np.ascontiguousarray((ln_g / 4.0).reshape(NH, 2, 128))[None],
    }

    xp = np.zeros((B, AP_, D), f32)
    xp[:, :A, :] = x
    xpb = xp.astype(bf16)

    in_maps = []
    for core in range(N_CORES):
        m = dict(shared)
        m["xb"] = np.ascontiguousarray(xpb[core * BL:(core + 1) * BL])
        in_maps.append(m)
    return in_maps


_NC_CACHE = {}


def kernel(**inputs):
    if "nc" not in _NC_CACHE:
        _NC_CACHE["nc"] = build()
    nc = _NC_CACHE["nc"]
    in_maps = host_prep(inputs)
    res = run_bass_kernel_spmd(nc, in_maps, core_ids=list(range(N_CORES)))
    out = np.concatenate([r["out"] for r in res.results], axis=0)
    out = out + np.asarray(inputs["out_b"], np.float32)[None, None, :]
    return out.astype(np.float32)


# revision 52
# speedup vs baseline: 1.0014x; 1.0011x over previous
"""Trainium2 Bass kernel for nn_DistributedDotGAT (B=32, A=100, D=10000).

Sharding: data-parallel over batch across 8 cores (4 batches/core), params
replicated. Per-core pipeline:
  A. ragged gather: DVE prefix-scan ranks + GPSIMD local_scatter compaction
     (mask/rank/idx all on DVE; coordinate decode via fmod on DVE)
  B. entry encoder (fourier features via PE + fused fmod range-reduction +
     Sin on ACT; enc bias folded into a ones feature row; enc layer-2 folded
     into comp_W1 on host) and per-agent compression with streamed bf16
     weights accumulating in PSUM
  C. 3 rounds of multi-head dot-product attention (qW^T kW folded on host so
     only one score-side projection is needed; alpha transposed via XBAR DMA)
  D. output projection: bf16 out_W prefetched during C, PSUM->SBUF batched
     copies, one batched store DMA per column chunk on the gpsimd queue
"""
import sys
import math
import numpy as np

for _p in ("/opt/trn_rl_repo", "/root/.axon_site/_ro/trn_rl_repo"):
    if _p not in sys.path:
        sys.path.insert(0, _p)

import ml_dtypes
import concourse.bass as bass
import concourse.bacc as bacc
import concourse.tile as tile
import concourse.mybir as mybir
from concourse import library_config
from concourse.bass_utils import run_bass_kernel_spmd

dt = mybir.dt
Alu = mybir.AluOpType
Act = mybir.ActivationFunctionType
Ax = mybir.AxisListType

N_CORES = 8
B, A, D = 32, 100, 10000
HID, NH, OUT, NFREQ = 256, 4, 10000, 16
E = 100          # max entries kept per (b, agent)
NGRID = 100      # row/col decode base
BL = B // N_CORES   # 4 batches per core
NPAIR = BL * A      # 400 entry columns per slot
AP_ = 112        # padded agent/partition count
STEPS = 3
TWO_PI = 2.0 * math.pi
NSLOT_CHUNK = 8
NCHUNK = AP_ // NSLOT_CHUNK          # 14 chunks of 8 slots
ENT_C = NSLOT_CHUNK * NPAIR          # 3200 entries per slot-chunk
GK = 4                               # G slots per DMA batch
OUTC = 512                           # out-proj free chunk
MAGIC = 12582912.0   # 1.5 * 2**23, fp32 round-to-int magic constant
bf16 = ml_dtypes.bfloat16


def build(debug=False):
    nc = bacc.Bacc("TRN2", target_bir_lowering=False, debug=False,
                   num_devices=N_CORES)

    def din(name, shape, dtype):
        return nc.dram_tensor(name, shape, dtype, kind="ExternalInput")

    xb = din("xb", [BL, AP_, D], dt.bfloat16)
    ptab = din("ptab", [1, D], dt.uint16)
    sliota = din("sliota", [AP_, AP_], dt.float32)
    identf = din("identf", [128, 128], dt.float32r)
    identb = din("identb", [128, 128], dt.bfloat16)
    onesrow = din("onesrow", [1, ENT_C], dt.float32r)
    dupB4 = din("dupB4", [4, 32], dt.float32r)
    addp = din("addp", [32, 1], dt.float32)
    sbp = din("sbp", [32, 1], dt.float32)
    w1pos = din("w1pos", [34, 256], dt.float32r)
    gw = din("gw", [E // GK, 128, GK, 2, 512], dt.bfloat16)
    corrv = din("corrv", [AP_, 512], dt.bfloat16)
    cb1c = din("cb1c", [128, 4], dt.float32)
    w2t = din("w2t", [128, 4, 256], dt.float32r)
    b2c = din("b2c", [128, 2], dt.float32)
    mw = din("mw", [128, NH, 2, 256], dt.float32r)
    vw = din("vw", [128, NH, 2, 256], dt.float32r)
    fpw = din("fpw", [128, 2, NH, 2, 256], dt.float16)
    fpbr = din("fpbr", [1, 2, NH, 2, 128], dt.float32r)
    lngb = din("lngb", [128, 2, 2, NH], dt.float32)
    connc = din("connc", [A, A], dt.float32)
    outwtb = din("outwtb", [128, 2, OUT], dt.float16)
    onesmat = din("onesmat", [128, 128], dt.float32r)
    lngr = din("lngr", [1, NH, 2, 128], dt.float32r)

    out = nc.dram_tensor("out", [BL, A, OUT], dt.float32, kind="ExternalOutput")

    with tile.TileContext(nc) as tc:
        nc.gpsimd.load_library(library_config.local_scatter)

        # Chain ACT ops in emission order so the scheduler cannot interleave
        # activations from different pwp table sets (each switch costs 1.3us).
        _last_act = [None]

        def act(*args, **kw):
            return nc.scalar.activation(*args, **kw)

        with tc.tile_pool(name="const", bufs=1) as cpool:
            sliota_t = cpool.tile([AP_, AP_], dt.float32, tag="sliota")
            nc.scalar.dma_start(sliota_t[:], sliota.ap())
            dupB4_t = cpool.tile([4, 32], dt.float32r, tag="dupB4")
            nc.scalar.dma_start(dupB4_t[:], dupB4.ap())
            addp_t = cpool.tile([32, 1], dt.float32, tag="addp")
            nc.scalar.dma_start(addp_t[:], addp.ap())
            sbp_t = cpool.tile([32, 1], dt.float32, tag="sbp")
            nc.scalar.dma_start(sbp_t[:], sbp.ap())
            w1pos_t = cpool.tile([34, 256], dt.float32r, tag="w1pos")
            nc.scalar.dma_start(w1pos_t[:], w1pos.ap())
            corrv_t = cpool.tile([AP_, 512], dt.bfloat16, tag="corrv")
            nc.scalar.dma_start(corrv_t[:], corrv.ap())
            cb1c_t = cpool.tile([128, 4], dt.float32, tag="cb1c")
            nc.scalar.dma_start(cb1c_t[:], cb1c.ap())
            w2t_t = cpool.tile([128, 4, 256], dt.float32r, tag="w2t")
            nc.scalar.dma_start(w2t_t[:], w2t.ap())
            b2c_t = cpool.tile([128, 2], dt.float32, tag="b2c")
            nc.scalar.dma_start(b2c_t[:], b2c.ap())
            cap_t = cpool.tile([AP_, 1], dt.float32, tag="cap")
            nc.vector.memset(cap_t[:], 255.0)
            eps_t = cpool.tile([128, 1], dt.float32, tag="eps")
            nc.vector.memset(eps_t[:], 1e-5)
            cnt_t = cpool.tile([AP_, BL], dt.float32, tag="cnt")
            # transposed entry arrays ([slot, b, pair]), persist into phase B
            # rcT packs row*256 (rc=0) and col (rc=1) for a single rcflat DMA
            rcT = cpool.tile([AP_, 2, NPAIR], dt.float32r, tag="rcT")
            valTall = cpool.tile([AP_, NPAIR], dt.float32r, tag="valT")
            invTall = cpool.tile([AP_, NPAIR], dt.bfloat16, tag="invT")
            # h ping-pong tiles (persist across phases)
            htiles = [cpool.tile([128, 2, NPAIR], dt.float32r,
                                 name=f"hst{i}", tag=f"hst{i}")
                      for i in range(2)]

            # ------------- Phase A: scan + scatter + transpose, per b -------
            with tc.tile_pool(name="pA", bufs=2) as pA, \
                 tc.tile_pool(name="pA1", bufs=1) as pA1, \
                 tc.tile_pool(name="psA", bufs=2,
                              space=bass.MemorySpace.PSUM) as psA:
                identf_t = pA1.tile([128, 128], dt.float32r, tag="identf")
                nc.scalar.dma_start(identf_t[:], identf.ap())
                identb_t = pA1.tile([128, 128], dt.bfloat16, tag="identb")
                nc.scalar.dma_start(identb_t[:], identb.ap())
                ptab_t = pA1.tile([AP_, D], dt.uint16, tag="ptab")
                nc.gpsimd.dma_start(ptab_t[:],
                                    ptab.ap().broadcast_to([AP_, D]))
                DC = 2500
                NJ = D // DC
                for b in range(BL):
                    dvps = []
                    dpps = []
                    ct_prev = None
                    for j in range(NJ):
                        dsl = slice(j * DC, (j + 1) * DC)
                        xt = pA.tile([AP_, DC], dt.bfloat16, tag="xt", bufs=3,
                                     name="xt")
                        nc.sync.dma_start(xt[:], xb.ap()[b, :, dsl])
                        mk = pA.tile([AP_, DC], dt.bfloat16, tag="mk", bufs=3,
                                     name="mk")
                        nc.vector.tensor_scalar(mk[:], xt[:], 0.0, None,
                                                Alu.not_equal)
                        ct = pA.tile([AP_, DC], dt.bfloat16, tag="ct", bufs=3,
                                     name="ct")
                        nc.vector.tensor_tensor_scan(
                            ct[:], mk[:], cap_t[:].broadcast_to([AP_, DC]),
                            0.0 if j == 0 else ct_prev[:, DC - 1:DC],
                            Alu.add, Alu.min)
                        ct_prev = ct
                        # sg = 300*mk - 301 (ACT keeps DVE free for the scan)
                        sg = pA.tile([AP_, DC], dt.bfloat16, tag="sg", bufs=3,
                                     name="sg")
                        act(sg[:], mk[:], Act.Copy, scale=300.0, bias=-301.0)
                        # idx = ct + sg  (= ct-1 for valid, out-of-range else)
                        idx16 = pA.tile([AP_, DC], dt.int16, tag="idx", bufs=3,
                                        name="idx16")
                        nc.vector.tensor_tensor(idx16[:], ct[:], sg[:],
                                                Alu.add)
                        dvp = pA.tile([AP_, 256], dt.bfloat16, tag=f"dvp{j}",
                                      name=f"dvp{j}")
                        nc.gpsimd.local_scatter(dvp[:], xt[:], idx16[:],
                                                channels=AP_, num_elems=256,
                                                num_idxs=DC)
                        dpp = pA.tile([AP_, 256], dt.uint16, tag=f"dpp{j}",
                                      name=f"dpp{j}")
                        nc.gpsimd.local_scatter(dpp[:], ptab_t[:, dsl],
                                                idx16[:], channels=AP_,
                                                num_elems=256, num_idxs=DC)
                        dvps.append(dvp)
                        dpps.append(dpp)
                    nc.vector.tensor_copy(cnt_t[:, b:b + 1],
                                          ct_prev[:, DC - 1:DC])
                    # merge chunk scatters (disjoint support)
                    va = pA.tile([AP_, 256], dt.bfloat16, tag="va", name="va")
                    vb = pA.tile([AP_, 256], dt.bfloat16, tag="vb", name="vb")
                    nc.vector.tensor_tensor(va[:], dvps[0][:], dvps[1][:],
                                            Alu.add)
                    nc.vector.tensor_tensor(vb[:], dvps[2][:], dvps[3][:],
                                            Alu.add)
                    dval = pA.tile([AP_, 256], dt.bfloat16, tag="dval",
                                   name="dval")
                    nc.vector.tensor_tensor(dval[:], va[:], vb[:], Alu.add)
                    pa = pA.tile([AP_, 256], dt.float32, tag="pa", name="pa")
                    pb_ = pA.tile([AP_, 256], dt.float32, tag="pb", name="pb")
                    nc.vector.tensor_tensor(pa[:], dpps[0][:], dpps[1][:],
                                            Alu.add)
                    nc.vector.tensor_tensor(pb_[:], dpps[2][:], dpps[3][:],
                                            Alu.add)
                    packf = pA.tile([AP_, 256], dt.float32, tag="packf",
                                    name="packf")
                    nc.vector.tensor_tensor(packf[:], pa[:], pb_[:], Alu.add)

                    # invalid-slot grid: (slot >= count)
                    invg = pA.tile([AP_, AP_], dt.bfloat16, tag="invg",
                                   name="invg")
                    nc.vector.tensor_scalar(invg[:], sliota_t[:],
                                            cnt_t[:, b:b + 1], None, Alu.is_ge)
                    # coords decode (DVE magic round): row = rnd(packf/256-.5)
                    rowt = pA.tile([AP_, AP_], dt.float32, tag="rowt",
                                   name="rowt")
                    nc.vector.tensor_scalar(rowt[:], packf[:, :AP_],
                                            2.0 ** -8, -0.498046875,
                                            Alu.mult, Alu.add)
                    rowf = pA.tile([AP_, AP_], dt.float32r, tag="rowf",
                                   name="rowf")
                    nc.vector.tensor_scalar(rowf[:], rowt[:], MAGIC, -MAGIC,
                                            Alu.add, Alu.add)
                    colf = pA.tile([AP_, AP_], dt.float32r, tag="colf",
                                   name="colf")
                    nc.vector.scalar_tensor_tensor(colf[:], rowf[:], -256.0,
                                                   packf[:, :AP_], Alu.mult,
                                                   Alu.add)

                    # transposes -> [slot, pair] layout
                    tv = psA.tile([AP_, AP_], dt.bfloat16, tag="tv", name="tv")
                    nc.tensor.transpose(tv[:], dval[:, :AP_],
                                        identb_t[:AP_, :AP_])
                    nc.vector.tensor_copy(valTall[:, b * A:(b + 1) * A],
                                          tv[:, :A])
                    tr_ = psA.tile([AP_, AP_], dt.float32r, tag="tr",
                                   name="tr_")
                    nc.tensor.transpose(tr_[:], rowf[:], identf_t[:AP_, :AP_])
                    nc.vector.tensor_copy(rcT[:, 0, b * A:(b + 1) * A],
                                          tr_[:, :A])
                    tcl = psA.tile([AP_, AP_], dt.float32r, tag="tc",
                                   name="tcl")
                    nc.tensor.transpose(tcl[:], colf[:], identf_t[:AP_, :AP_])
                    nc.vector.tensor_copy(rcT[:, 1, b * A:(b + 1) * A],
                                          tcl[:, :A])
                    ti = psA.tile([AP_, AP_], dt.bfloat16, tag="ti", name="ti")
                    nc.tensor.transpose(ti[:], invg[:], identb_t[:AP_, :AP_])
                    nc.vector.tensor_copy(invTall[:, b * A:(b + 1) * A],
                                          ti[:, :A])

            # phase C weights: loaded once phase A's queue traffic is done
            onesmat_t = cpool.tile([128, 128], dt.float32r, tag="onesmat")
            nc.scalar.dma_start(onesmat_t[:], onesmat.ap())
            mw_t = cpool.tile([128, NH, 2, 256], dt.float32r, tag="mw")
            nc.scalar.dma_start(mw_t[:], mw.ap())
            vw_t = cpool.tile([128, NH, 2, 256], dt.float32r, tag="vw")
            nc.scalar.dma_start(vw_t[:], vw.ap())
            fpw_t = cpool.tile([128, 2, NH, 2, 256], dt.float16, tag="fpw")
            nc.scalar.dma_start(fpw_t[:], fpw.ap())
            fpbr_t = cpool.tile([1, 2, NH, 2, 128], dt.float32r, tag="fpbr")
            nc.scalar.dma_start(fpbr_t[:], fpbr.ap())
            ones4_t = cpool.tile([1, 512], dt.float32r, tag="ones4")
            nc.scalar.dma_start(ones4_t[:], onesrow.ap()[0:1, 0:512])
            lngb_t = cpool.tile([128, 2, 2, NH], dt.float32, tag="lngb")
            nc.scalar.dma_start(lngb_t[:], lngb.ap())
            connc_t = cpool.tile([A, A], dt.float32, tag="connc")
            nc.scalar.dma_start(connc_t[:], connc.ap())
            lngr_t = cpool.tile([1, NH, 2, 128], dt.float32r, tag="lngr")
            nc.scalar.dma_start(lngr_t[:], lngr.ap())

            # ------------- Phase B: entry encoder + compression -------------
            with tc.tile_pool(name="pB", bufs=3) as pB, \
                 tc.tile_pool(name="psH", bufs=1,
                              space=bass.MemorySpace.PSUM) as psH, \
                 tc.tile_pool(name="psZ", bufs=2,
                              space=bass.MemorySpace.PSUM) as psZ, \
                 tc.tile_pool(name="psP", bufs=2,
                              space=bass.MemorySpace.PSUM) as psP:
                h1ps = [psH.tile([128, NPAIR], dt.float32,
                                 name=f"h1_{mq}", tag=f"h1_{mq}")
                        for mq in range(4)]
                first = True

                chunks = {}
                rcflats = {}
                ntrCs = {}

                def rcflat_dma(c):
                    if c >= NB:
                        return
                    k0 = c * NSLOT_CHUNK
                    ksl = slice(k0, k0 + NSLOT_CHUNK)
                    rcflat = pB.tile([4, NSLOT_CHUNK, NPAIR], dt.float32r,
                                     tag="rcflat", bufs=4, name="rcflat")
                    for rr in range(4):
                        nc.sync.dma_start(rcflat[rr:rr + 1],
                                          rcT[ksl, rr % 2])
                    rcflats[c] = rcflat

                def featc_dma(c):
                    if c >= NB:
                        return
                    k0 = c * NSLOT_CHUNK
                    ksl = slice(k0, k0 + NSLOT_CHUNK)
                    featC = pB.tile([34, NSLOT_CHUNK, NPAIR], dt.float32r,
                                    tag="featC", name="featC")
                    nc.sync.dma_start(featC[32:33], valTall[ksl])
                    nc.sync.dma_start(featC[33:34], onesrow.ap())
                    chunks[c] = featC

                def proj_mm(c):
                    k0 = c * NSLOT_CHUNK
                    rcflat = rcflats.pop(c)
                    ntrC = pB.tile([32, NSLOT_CHUNK, NPAIR], dt.float16,
                                   tag="ntrC", bufs=2, name="ntrC")
                    nslot = min(NSLOT_CHUNK, E - k0)
                    for ks in range(nslot):
                        proj = psP.tile([32, NPAIR], dt.float32,
                                        tag="proj", name="proj")
                        nc.tensor.matmul(proj[:], dupB4_t[:], rcflat[:, ks, :],
                                         start=True, stop=True)
                        # magic round on DVE: u2 = rnd(proj+shift) + MAGIC
                        u2 = pB.tile([32, NPAIR], dt.float32, tag="u2",
                                     bufs=1, name="u2")
                        nc.vector.tensor_scalar(u2[:], proj[:], addp_t[:],
                                                MAGIC, Alu.add, Alu.add)
                        # ntr = rnd(proj+shift) - proj
                        nc.vector.scalar_tensor_tensor(
                            ntrC[:, ks, :], u2[:], -MAGIC, proj[:],
                            Alu.add, Alu.subtract)
                    ntrCs[c] = ntrC

                def proj_sin(c, half):
                    # sin(-2pi*ntr + 2pi*shift) = sin(2pi*(x+shift)); emitted
                    # in halves interleaved between zcomp silus so the ACT
                    # queue reaches them before the chunk boundary
                    k0 = c * NSLOT_CHUNK
                    nslot = min(NSLOT_CHUNK, E - k0)
                    h = (nslot + 1) // 2
                    lo, hi = (0, h) if half == 0 else (h, nslot)
                    if lo >= hi:
                        return
                    ntrC = ntrCs[c]
                    featC = chunks[c]
                    act(featC[:32, lo:hi, :], ntrC[:, lo:hi, :], Act.Sin,
                        scale=-TWO_PI, bias=sbp_t[:])

                def zcomp(c, mid=None):
                    """zpass for chunk c+1 interleaved with comppass for c."""
                    nonlocal first
                    nz = min(NSLOT_CHUNK, E - (c + 1) * NSLOT_CHUNK) \
                        if c + 1 < NB else 0
                    ncp = min(NSLOT_CHUNK, E - c * NSLOT_CHUNK) if c >= 0 else 0
                    sts_new = []
                    for ks in range(NSLOT_CHUNK):
                        if mid is not None and ks in (3, 6):
                            mid(0 if ks == 3 else 1)
                        if ks < nz:
                            featC = chunks[c + 1]
                            s_t = pB.tile([128, 2, NPAIR], dt.bfloat16,
                                          tag="s", bufs=2 * NSLOT_CHUNK,
                                          name="s")
                            for mh in range(2):
                                zb = psZ.tile([128, 512], dt.float32,
                                              tag="zb", name="zb")
                                nc.tensor.matmul(
                                    zb[:, :NPAIR],
                                    w1pos_t[:, mh * 128:(mh + 1) * 128],
                                    featC[:, ks, :], start=True, stop=True)
                                act(s_t[:, mh, :], zb[:, :NPAIR], Act.Silu)
                            sts_new.append(s_t)
                        # comppass for chunk c, slot ks
                        if ks < ncp:
                            k = c * NSLOT_CHUNK + ks
                            if k % GK == 0:
                                zcomp.gk4 = pB.tile([128, GK, 2, 512],
                                                    dt.bfloat16, tag="gk",
                                                    bufs=2, name="gk")
                                nc.scalar.dma_start(zcomp.gk4[:],
                                                    gw.ap()[k // GK])
                            gk4 = zcomp.gk4
                            s_t = zcomp.sts[ks]
                            for kh in range(2):
                                for mq in range(4):
                                    nc.tensor.matmul(
                                        h1ps[mq][:],
                                        gk4[:, k % GK, kh,
                                            mq * 128:(mq + 1) * 128],
                                        s_t[:, kh, :],
                                        start=first, stop=False,
                                        skip_group_check=True)
                                first = False
                    zcomp.sts = sts_new

                NB = (E + NSLOT_CHUNK - 1) // NSLOT_CHUNK
                for cc in range(4):
                    rcflat_dma(cc)
                featc_dma(0)
                featc_dma(1)
                proj_mm(0)
                proj_sin(0, 0)
                proj_sin(0, 1)
                featc_dma(2)
                zcomp(-1, mid=lambda half: (proj_mm(1) if half == 0 else None,
                                            proj_sin(1, half))
                      )     # prime: zpass chunk 0; chunk-1 skeleton mid-loop
                for c in range(NB):
                    rcflat_dma(c + 4)
                    featc_dma(c + 3)
                    if c + 2 < NB:
                        proj_mm(c + 2)
                    # comp chunk c + zpass chunk c+1; sin halves for c+2
                    # interleave mid-loop so ACT reaches them early
                    zcomp(c, mid=(lambda half, cc=c:
                                  proj_sin(cc + 2, half))
                          if c + 2 < NB else None)

                # pad-token corrections + comp layer 2
                for mq in range(4):
                    nc.tensor.matmul(h1ps[mq][:],
                                     corrv_t[:, mq * 128:(mq + 1) * 128],
                                     invTall[:], start=False, stop=True,
                                     skip_group_check=True)
                hsw = pB.tile([128, 4, NPAIR], dt.float32r, tag="hsw",
                              bufs=1)
                for mq in range(4):
                    act(hsw[:, mq, :], h1ps[mq][:], Act.Silu,
                        bias=cb1c_t[:, mq:mq + 1])
                for ih in range(2):
                    zb2 = psZ.tile([128, 512], dt.float32, tag="zb",
                                   name="zb2")
                    for kq in range(4):
                        nc.tensor.matmul(zb2[:, :NPAIR],
                                         w2t_t[:, kq, ih * 128:(ih + 1) * 128],
                                         hsw[:, kq, :],
                                         start=(kq == 0), stop=(kq == 3))
                    act(htiles[0][:, ih, :], zb2[:, :NPAIR], Act.Identity,
                        bias=b2c_t[:, ih:ih + 1])

            # ------------- Phase C: GAT message passing ---------------------
            with tc.tile_pool(name="pD0", bufs=1) as pD0:
                # bf16 out_W, prefetched in chunks between attention steps
                woutb = pD0.tile([128, 2, OUT], dt.float16, tag="woutb")
                WCH = OUT // 8

                def wout_prefetch(i):
                    sl = slice(i * WCH, (i + 1) * WCH)
                    nc.sync.dma_start(woutb[:, :, sl], outwtb.ap()[:, :, sl])

                phaseC = tc.tile_pool(name="pC1", bufs=1)
                pC1 = phaseC.__enter__()
                pC2 = tc.tile_pool(name="pC2", bufs=3).__enter__()
                psQ = tc.tile_pool(name="psQ", bufs=3,
                                   space=bass.MemorySpace.PSUM).__enter__()
                psS = tc.tile_pool(name="psS", bufs=2,
                                   space=bass.MemorySpace.PSUM).__enter__()

                hcur = htiles[0]
                for step in range(STEPS):
                    wout_prefetch(step)
                    # bf16 copy of h shared by all heads (scores rhs)
                    hb = pC2.tile([128, 2, NPAIR], dt.bfloat16, tag="hb",
                                  name="hb")
                    for kh in range(2):
                        nc.vector.tensor_copy(hb[:, kh, :], hcur[:, kh, :])
                    Rbs, VTs, aTs = [], [], []
                    # --- R = (qW^T kW / 16)^T-projected h;  V ---
                    for n in range(NH):
                        rp = psQ.tile([128, 2, 512], dt.float32, tag="q2",
                                      name="rp")
                        for jh in range(2):
                            for kh in range(2):
                                nc.tensor.matmul(
                                    rp[:, jh, :NPAIR],
                                    mw_t[:, n, kh, jh * 128:(jh + 1) * 128],
                                    hcur[:, kh, :],
                                    start=(kh == 0), stop=(kh == 1))
                        rb = pC1.tile([128, 2, NPAIR], dt.bfloat16,
                                      tag=f"rb{n}", name=f"rb{n}")
                        nc.vector.tensor_copy(rb[:], rp[:, :, :NPAIR])
                        Rbs.append(rb)
                        vp = psQ.tile([128, 2, 512], dt.float32, tag="q2",
                                      name="vp")
                        for b in range(BL):
                            for kh in range(2):
                                nc.tensor.matmul(
                                    vp[:A, b // 2, (b % 2) * 256:
                                       (b % 2) * 256 + 256],
                                    hcur[:, kh, b * A:(b + 1) * A],
                                    vw_t[:, n, kh, :],
                                    start=(kh == 0), stop=(kh == 1))
                        vt_t = pC1.tile([AP_, BL, 256], dt.bfloat16,
                                        tag=f"vts{n}", name=f"vts{n}")
                        nc.vector.tensor_copy(
                            vt_t[:A].reshape([A, 2, 2, 256]),
                            vp[:A].reshape([A, 2, 2, 256]))
                        VTs.append(vt_t)
                    # --- scores + softmax, batched over b per head ---
                    for n in range(NH):
                        scp = psS.tile([A, 512], dt.float32, tag="sc",
                                       name="scp")
                        for b in range(BL):
                            for kh in range(2):
                                nc.tensor.matmul(
                                    scp[:, b * 128:b * 128 + A],
                                    Rbs[n][:, kh, b * A:(b + 1) * A],
                                    hb[:, kh, b * A:(b + 1) * A],
                                    start=(kh == 0), stop=(kh == 1))
                        sca = pC2.tile([A, BL, A], dt.float32, tag="sca",
                                       name="sca")
                        nc.vector.tensor_tensor(
                            sca[:],
                            scp[:].reshape([A, BL, 128])[:, :, :A],
                            connc_t[:].reshape([A, 1, A])
                            .broadcast_to([A, BL, A]),
                            Alu.add)
                        esc = pC2.tile([A, BL, A], dt.float32, tag="esc",
                                       name="esc")
                        act(esc[:], sca[:], Act.Exp)
                        sm = pC2.tile([A, BL], dt.float32, tag="sm", name="sm")
                        nc.vector.tensor_reduce(sm[:], esc[:], Ax.X, Alu.add)
                        rs = pC2.tile([A, BL], dt.float32, tag="rs", name="rs")
                        nc.vector.reciprocal(rs[:], sm[:])
                        alp = pC2.tile([128, BL, 128], dt.bfloat16, tag="alp",
                                       name="alp")
                        nc.vector.tensor_tensor(
                            alp[:A, :, :A], esc[:],
                            rs[:].reshape([A, BL, 1]).broadcast_to([A, BL, A]),
                            Alu.mult)
                        at_t = pC1.tile([128, BL, 128], dt.bfloat16,
                                        tag=f"ats{n}", name=f"ats{n}")
                        for b in range(BL):
                            nc.sync.dma_start_transpose(at_t[:, b, :],
                                                        alp[:, b, :])
                        aTs.append(at_t)
                    # --- message + MLP + LN stats ---
                    ms, vvs, tss = [], [], []
                    for n in range(NH):
                        hmp = psQ.tile([128, 2, 512], dt.float32, tag="q2",
                                       name="hmp")
                        for jh in range(2):
                            for b in range(BL):
                                nc.tensor.matmul(
                                    hmp[:, jh, b * A:(b + 1) * A],
                                    VTs[n][:A, b, jh * 128:(jh + 1) * 128],
                                    aTs[n][:A, b, :A],
                                    start=True, stop=True,
                                    skip_group_check=True)
                        hs_t = pC2.tile([128, 2, NPAIR], dt.bfloat16,
                                        tag="hs")
                        act(hs_t[:], hmp[:, :, :NPAIR], Act.Silu)
                        t1p = psQ.tile([128, 2, 512], dt.float32, tag="q2",
                                       name="t1p")
                        for ih in range(2):
                            for jh in range(2):
                                nc.tensor.matmul(
                                    t1p[:, ih, :NPAIR],
                                    fpw_t[:, 0, n, jh, ih * 128:(ih + 1) * 128],
                                    hs_t[:, jh, :],
                                    start=(jh == 0), stop=(jh == 1))
                        t1s = pC2.tile([128, 2, NPAIR], dt.bfloat16, tag="t1s")
                        for ih in range(2):
                            act(t1s[:, ih, :], t1p[:, ih, :NPAIR], Act.Silu,
                                bias=fpb_t[:, 0, ih, n:n + 1])
                        t2p = psQ.tile([128, 2, 512], dt.float32, tag="q2",
                                       name="t2p")
                        for ih in range(2):
                            for jh in range(2):
                                nc.tensor.matmul(
                                    t2p[:, ih, :NPAIR],
                                    fpw_t[:, 1, n, jh, ih * 128:(ih + 1) * 128],
                                    t1s[:, jh, :],
                                    start=(jh == 0), stop=(jh == 1))
                        ts_t = pC1.tile([128, 2, NPAIR], dt.float32r,
                                        tag=f"ts{n}")
                        for ih in range(2):
                            act(ts_t[:, ih, :], t2p[:, ih, :NPAIR],
                                Act.Identity, bias=fpb_t[:, 1, ih, n:n + 1])
                        tsq = pC2.tile([128, 2, NPAIR], dt.float32r, tag="tsq")
                        nc.vector.tensor_tensor(tsq[:], ts_t[:], ts_t[:],
                                                Alu.mult)
                        mtp = psS.tile([A, 512], dt.float32, tag="sc",
                                       name="mtp")
                        for ih in range(2):
                            nc.tensor.matmul(mtp[:1, :NPAIR],
                                             onesmat_t[:, 0:1],
                                             ts_t[:, ih, :],
                                             start=(ih == 0), stop=(ih == 1))
                        vtp = psS.tile([A, 512], dt.float32, tag="sc",
                                       name="vtp")
                        for ih in range(2):
                            nc.tensor.matmul(vtp[:1, :NPAIR],
                                             onesmat_t[:, 0:1],
                                             tsq[:, ih, :],
                                             start=(ih == 0), stop=(ih == 1))
                        m_t = pC1.tile([1, NPAIR], dt.float32r, tag=f"m{n}")
                        act(m_t[:], mtp[:1, :NPAIR], Act.Identity,
                            scale=1.0 / 256.0)
                        msq = pC2.tile([1, NPAIR], dt.float32r, tag="msq")
                        nc.vector.tensor_tensor(msq[:], m_t[:], m_t[:],
                                                Alu.mult)
                        vv = pC1.tile([1, NPAIR], dt.float32, tag=f"vv{n}")
                        nc.vector.scalar_tensor_tensor(
                            vv[:], vtp[:1, :NPAIR], 1.0 / 256.0, msq[:],
                            Alu.mult, Alu.subtract)
                        ms.append(m_t); vvs.append(vv); tss.append(ts_t)
                    # --- rstd = exp(-0.5*ln(v+eps)) (one ln+exp table set) ---
                    rstds = []
                    lnvs = []
                    for n in range(NH):
                        lnv = pC1.tile([1, NPAIR], dt.float32, tag=f"lnv{n}",
                                       name=f"lnv{n}")
                        act(lnv[:], vvs[n][:], Act.Ln, bias=eps_t[:1])
                        lnvs.append(lnv)
                    for n in range(NH):
                        rstd = pC1.tile([1, NPAIR], dt.float32r,
                                        tag=f"rsd{n}", name=f"rsd{n}")
                        act(rstd[:], lnvs[n][:], Act.Exp, scale=-0.5)
                        rstds.append(rstd)
                    # --- hnew = sum_n ts_n*(g*rstd)_n + bsum - sum_n(g*m*rstd)
                    hnew = htiles[(step + 1) % 2]
                    mrs = []
                    for n in range(NH):
                        mr = pC1.tile([1, NPAIR], dt.float32r, tag=f"mr{n}",
                                      name=f"mr{n}")
                        nc.vector.tensor_tensor(mr[:], ms[n][:], rstds[n][:],
                                                Alu.mult)
                        mrs.append(mr)
                    for ih in range(2):
                        mgp = psQ.tile([128, 2, 512], dt.float32, tag="q2",
                                       name="mgp")
                        for n in range(NH):
                            nc.tensor.matmul(mgp[:, 0, :NPAIR],
                                             lngr_t[0:1, n, ih, :],
                                             mrs[n][:], start=(n == 0),
                                             stop=(n == 3))
                        for n in range(NH):
                            nc.tensor.matmul(mgp[:, 1, :NPAIR],
                                             lngr_t[0:1, n, ih, :],
                                             rstds[n][:], start=(n == 0),
                                             stop=(n == 3),
                                             skip_group_check=True)
                            if n == 0:
                                nc.vector.tensor_tensor(hnew[:, ih, :],
                                                        tss[n][:, ih, :],
                                                        mgp[:, 1, :NPAIR],
                                                        Alu.mult)
                            else:
                                u1 = pC2.tile([128, NPAIR], dt.float32,
                                              tag="u1", name="u1")
                                nc.vector.tensor_tensor(u1[:],
                                                        tss[n][:, ih, :],
                                                        mgp[:, 1, :NPAIR],
                                                        Alu.mult)
                                nc.vector.tensor_tensor(hnew[:, ih, :],
                                                        hnew[:, ih, :], u1[:],
                                                        Alu.add)
                        nc.vector.scalar_tensor_tensor(
                            hnew[:, ih, :], hnew[:, ih, :],
                            lngb_t[:, 1, ih, 0:1], mgp[:, 0, :NPAIR],
                            Alu.add, Alu.subtract)
                    hcur = hnew
                wout_prefetch(3)

                # ------------- Phase D: output projection -------------------
                with tc.tile_pool(name="pD", bufs=3) as pD, \
                     tc.tile_pool(name="psD", bufs=2,
                                  space=bass.MemorySpace.PSUM) as psD:
                    hfb = pD.tile([128, 2, NPAIR], dt.float16, tag="hfb",
                                  bufs=1)
                    for ih in range(2):
                        nc.vector.tensor_copy(hfb[:, ih, :], hcur[:, ih, :])
                    for ci, c0 in enumerate(range(0, OUT, OUTC)):
                        w = min(OUTC, OUT - c0)
                        pop = psD.tile([A, BL, OUTC], dt.float32, tag="pop")
                        for b in range(BL):
                            for ih in range(2):
                                nc.tensor.matmul(pop[:, b, :w],
                                                 hfb[:, ih, b * A:(b + 1) * A],
                                                 woutb[:, ih, c0:c0 + w],
                                                 start=(ih == 0),
                                                 stop=(ih == 1),
                                                 skip_group_check=True)
                        ost = pD.tile([A, BL, OUTC], dt.float32, tag="ost")
                        if ci % 2 == 0:
                            act(ost[:], pop[:], Act.Copy)
                        else:
                            nc.vector.tensor_copy(ost[:], pop[:])
                        nc.gpsimd.dma_start(
                            out.ap()[:, :, c0:c0 + w]
                            .rearrange("b a c -> a b c"),
                            ost[:, :, :w])

    nc.compile()
    return nc


def host_prep(inputs):
    f32 = np.float32
    x = np.asarray(inputs["x"], f32)
    enc_W1 = np.asarray(inputs["enc_W1"], f32)
    enc_b1 = np.asarray(inputs["enc_b1"], f32)
    enc_W2 = np.asarray(inputs["enc_W2"], f32)
    enc_b2 = np.asarray(inputs["enc_b2"], f32)
    comp_W1 = np.asarray(inputs["comp_W1"], f32)
    comp_b1 = np.asarray(inputs["comp_b1"], f32)
    comp_W2 = np.asarray(inputs["comp_W2"], f32)
    comp_b2 = np.asarray(inputs["comp_b2"], f32)
    pad = np.asarray(inputs["pad_token"], f32)
    fB = np.asarray(inputs["fourier_B"], f32)
    qW = np.asarray(inputs["qW"], f32)
    kW = np.asarray(inputs["kW"], f32)
    vW = np.asarray(inputs["vW"], f32)
    fp_W1 = np.asarray(inputs["fp_W1"], f32)
    fp_b1 = np.asarray(inputs["fp_b1"], f32)
    fp_W2 = np.asarray(inputs["fp_W2"], f32)
    fp_b2 = np.asarray(inputs["fp_b2"], f32)
    ln_g = np.asarray(inputs["ln_g"], f32)
    ln_b = np.asarray(inputs["ln_b"], f32)
    conn = np.asarray(inputs["connectivity"], f32)
    out_W = np.asarray(inputs["out_W"], f32)

    M = comp_W1.reshape(512, E, HID)
    G = np.einsum('rkj,jl->rkl', M, enc_W2, optimize=True)      # [512, E, 256]
    feat0 = np.concatenate([[0.0], np.zeros(16, f32),
                            np.ones(16, f32)]).astype(f32)
    z00 = feat0 @ enc_W1.T + enc_b1
    e00 = (z00 / (1 + np.exp(-z00))) @ enc_W2.T + enc_b2
    corrV = np.einsum('rkj,j->rk', M, (pad - e00))               # [512, E]
    cb1p = comp_b1 + np.einsum('rkj,j->r', M, enc_b2)

    # gw[g, p, ki, kh, r] = G[r, g*GK+ki, kh*128+p]
    Gr = G.reshape(512, E // GK, GK, 2, 128)          # [r, g, ki, kh, p]
    gw = np.ascontiguousarray(Gr.transpose(1, 4, 2, 3, 0)).astype(bf16)

    corrv = np.zeros((AP_, 512), f32)
    corrv[:E] = corrV.T
    corrv = corrv.astype(bf16)

    # split fourier coefs: 10-bit-quantized high part (integer-exact products
    # through the f32r matmul) + small residual; row features carry row*256
    # so the row coefficients are pre-divided by 256 (exact in fp32)
    bhi = np.round(fB * 1024.0) / 1024.0
    blo = (fB - bhi).astype(f32)
    bhi = bhi.astype(f32)
    dupB4 = np.zeros((4, 32), f32)
    dupB4[0, :16] = bhi[:, 0]; dupB4[0, 16:] = bhi[:, 0]
    dupB4[1, :16] = bhi[:, 1]; dupB4[1, 16:] = bhi[:, 1]
    dupB4[2, :16] = blo[:, 0]; dupB4[2, 16:] = blo[:, 0]
    dupB4[3, :16] = blo[:, 1]; dupB4[3, 16:] = blo[:, 1]
    # per-partition phase shift (+0.25 on the cos half), applied before the
    # magic-round add so it survives fp32; matching sin-arg bias
    addp = np.zeros((32, 1), f32)
    addp[16:] = 0.25
    sbp = np.zeros((32, 1), f32)
    sbp[16:] = TWO_PI * 0.25

    w1pos = np.zeros((34, 256), f32)
    w1pos[:32] = enc_W1[:, 1:33].T
    w1pos[32] = enc_W1[:, 0]
    w1pos[33] = enc_b1

    cb1c = np.ascontiguousarray(cb1p.reshape(4, 128).T)
    w2t = np.ascontiguousarray(
        comp_W2.T.reshape(4, 128, 256).transpose(1, 0, 2))
    b2c = np.ascontiguousarray(comp_b2.reshape(2, 128).T)

    # fold qW^T kW (and the 1/sqrt(HID) score scale) into one matrix
    Mn = np.einsum('nji,njk->nik', qW * 0.25, kW * 0.25)   # [NH, 256, 256]
    mw = np.ascontiguousarray(
        Mn.reshape(NH, 2, 128, 256).transpose(2, 0, 1, 3))  # [128,NH,2,256]
    vw = np.ascontiguousarray(
        vW.transpose(0, 2, 1).reshape(NH, 2, 128, 256)
        .transpose(2, 0, 1, 3))                             # [128,NH,2,256]
    fpw = np.stack([fp_W1, fp_W2])                    # [2, n, i, j]
    fpw = fpw.transpose(0, 1, 3, 2).reshape(2, NH, 2, 128, 256)
    fpw = np.ascontiguousarray(fpw.transpose(3, 0, 1, 2, 4)).astype(np.float16)
    fpbr = np.ascontiguousarray(
        np.stack([fp_b1, fp_b2]).reshape(2, NH, 2, 128))[None]
    lngb = np.zeros((128, 2, 2, NH), f32)
    lg = (ln_g / 4.0).reshape(NH, 2, 128)             # [n, ih, p]
    lngb[:, 0, :, :] = lg.transpose(2, 1, 0)
    bsum = (ln_b / 4.0).sum(0).reshape(2, 128)        # [ih, p]
    lngb[:, 1, :, 0] = bsum.T

    outwtb = np.ascontiguousarray(
        out_W.T.reshape(2, 128, OUT).transpose(1, 0, 2)).astype(np.float16)

    ptab = (np.arange(D, dtype=np.uint32) // NGRID * 256
            + np.arange(D, dtype=np.uint32) % NGRID).astype(np.uint16)
    sliota = np.ascontiguousarray(
        np.broadcast_to(np.arange(AP_, dtype=f32)[None, :], (AP_, AP_)))
    identf = np.eye(128, dtype=f32)
    identb = np.eye(128, dtype=f32).astype(bf16)

    shared = {
        "ptab": ptab[None, :], "sliota": sliota, "identf": identf,
        "identb": identb, "onesrow": np.ones((1, ENT_C), f32),
        "dupB4": dupB4, "addp": addp, "sbp": sbp, "w1pos": w1pos, "gw": gw,
        "corrv": corrv, "cb1c": cb1c, "w2t": w2t, "b2c": b2c, "mw": mw,
        "vw": vw, "fpw": fpw, "fpbr": fpbr, "lngb": lngb,
        "connc": np.ascontiguousarray(conn), "outwtb": outwtb,
        "onesmat": np.ones((128, 128), f32),
        "lngr": np.ascontiguousarray((ln_g / 4.0).reshape(NH, 2, 128))[None],
    }

    xp = np.zeros((B, AP_, D), f32)
    xp[:, :A, :] = x
    xpb = xp.astype(bf16)

    in_maps = []
    for core in range(N_CORES):
        m = dict(shared)
        m["xb"] = np.ascontiguousarray(xpb[core * BL:(core + 1) * BL])
        in_maps.append(m)
    return in_maps


_NC_CACHE = {}


def kernel(**inputs):
    if "nc" not in _NC_CACHE:
        _NC_CACHE["nc"] = build()
    nc = _NC_CACHE["nc"]
    in_maps = host_prep(inputs)
    res = run_bass_kernel_spmd(nc, in_maps, core_ids=list(range(N_CORES)))
    out = np.concatenate([r["out"] for r in res.results], axis=0)
    out = out + np.asarray(inputs["out_b"], np.float32)[None, None, :]
    return out.astype(np.float32)
